# revision 26
# baseline (speedup 1.0000x reference)
"""Cross-attention Trainium2 kernel (8 NeuronCores, SPMD).

Sharding: core c handles batch c//2 and head-group c%2 (8 of 16 heads).
Each core computes its head-group's partial output projection; the host
sums the two partials per batch (bias is folded into head-group 0).

Shapes (hardcoded): B=4, N=2048 (queries), M=1024 (context), K=1024
(query/context dim), H=16 heads, DH=64, head-group width DHG=512, E=1024.

Dataflow (bf16 compute, fp32 PSUM accumulation / fp32 output):
  host pre-casts x/ctx/weights to bf16 and pre-broadcasts bias.
  xT/ctxT loaded via XBAR DMA transpose (no PE transposes for inputs).
  K.T = Wk.T @ ctxT, V = ctxT.T @ Wv, Q.T = Wq.T @ xT (bf16 matmuls).
  Per (q-tile of 128, head): S.T[m,q] (8 matmuls, d=64), P.T = exp on ACT
  (one 1024-wide activation per head), PV in [q-part, 65] orientation
  (V plus ones column -> softmax row sums land in column 64), reciprocal +
  per-partition-scalar normalize on DVE, O per q-tile PE-transposed back
  to O.T, out = O.T.T @ Wo; bias added by DVE during the PSUM->SBUF copy.
  The finalize work of each q-tile (transpose, output projection, store)
  and the next block's Q.T projection are split into ~450ns pieces and
  interleaved one-per-head into the following q-tile's S/exp/PV loop so
  PE stays busy during the exp latency of the S->exp->PV chain.
"""
import sys

if "/opt/trn_rl_repo" not in sys.path:
    sys.path.insert(0, "/opt/trn_rl_repo")

import numpy as np
import ml_dtypes

import concourse.bass as bass  # noqa: F401
import concourse.tile as tile
from concourse import bacc, mybir
from concourse.bass_utils import run_bass_kernel_spmd

P = 128
N = 2048          # queries per batch
M = 1024          # context rows
K = 1024          # query_dim == context_dim
DHG = 512         # d_attn per head group (8 heads x 64)
DH = 64           # dim per head
HL = 8            # heads per core
E = 1024          # output dim
SCALE = DH ** -0.5
F32 = mybir.dt.float32
BF16 = mybir.dt.bfloat16

KO = K // P       # 8 contraction chunks
NT = N // P       # 16 query tiles
MT = M // P       # 8 context tiles
DO = DHG // P     # 4 head-dim chunks
QC = N // 512     # 4 query blocks of 512
EC = E // 512     # 2 output chunks of 512

_CACHE = {}


def _build():
    nc = bacc.Bacc("TRN2", target_bir_lowering=False, debug=False, num_devices=8)
    x_d = nc.dram_tensor("x", [N, K], BF16, kind="ExternalInput")
    ctx_d = nc.dram_tensor("ctx", [M, K], BF16, kind="ExternalInput")
    wq_d = nc.dram_tensor("wq", [K, DHG], BF16, kind="ExternalInput")
    wk_d = nc.dram_tensor("wk", [K, DHG], BF16, kind="ExternalInput")
    wv_d = nc.dram_tensor("wv", [K, DHG], BF16, kind="ExternalInput")
    wo_d = nc.dram_tensor("wo", [DHG, E], BF16, kind="ExternalInput")
    bo_d = nc.dram_tensor("bo", [P, E], F32, kind="ExternalInput")
    id_d = nc.dram_tensor("ident", [P, P], F32, kind="ExternalInput")
    out_d = nc.dram_tensor("out", [N, E], F32, kind="ExternalOutput")

    with tile.TileContext(nc) as tc:
        with tc.tile_pool(name="persist", bufs=1) as pp:
            # DMA issue order tuned so K-proj (wk+ctxT) then Q-proj (wq+xT0)
            # inputs arrive first on the serialized DMA device.
            ident = pp.tile([P, P], F32)
            bo_sb = pp.tile([P, E], F32)
            wq_sb = pp.tile([P, KO, DHG], BF16)
            wk_sb = pp.tile([P, KO, DHG], BF16)
            wv_sb = pp.tile([P, KO, DHG], BF16)
            wo_sb = pp.tile([P, DO, E], BF16)
            ctxT = pp.tile([P, KO, M], BF16)
            xT = pp.tile([P, KO, N], BF16)

            nc.sync.dma_start(wk_sb[:], wk_d.rearrange("(ko p) d -> p ko d", p=P))
            for ms in range(2):
                nc.sync.dma_start_transpose(
                    ctxT[:, :, ms * 512:(ms + 1) * 512],
                    ctx_d[ms * 512:(ms + 1) * 512, :],
                )
            nc.sync.dma_start(wq_sb[:], wq_d.rearrange("(ko p) d -> p ko d", p=P))
            nc.sync.dma_start_transpose(xT[:, :, 0:512], x_d[0:512, :])
            nc.sync.dma_start(wv_sb[:], wv_d.rearrange("(ko p) d -> p ko d", p=P))
            nc.sync.dma_start(wo_sb[:], wo_d.rearrange("(do p) e -> p do e", p=P))
            for ns in range(1, QC):
                nc.sync.dma_start_transpose(
                    xT[:, :, ns * 512:(ns + 1) * 512],
                    x_d[ns * 512:(ns + 1) * 512, :],
                )
            nc.sync.dma_start(ident[:], id_d[:])
            nc.sync.dma_start(bo_sb[:], bo_d[:])

            kT = pp.tile([P, DO, M], BF16)        # K.T  [dhg, m]
            qT = pp.tile([P, DO, N], BF16)        # Q.T  [dhg, n]
            v_sb = pp.tile([P, MT, HL, DH + 1], BF16)  # V + ones col per head
            oT_sb = pp.tile([P, DO, N], BF16)     # O.T  [dhg, n] normalized
            nc.vector.memset(v_sb[:, :, :, DH], 1.0)

            with tc.tile_pool(name="psBig", bufs=2, space="PSUM") as psB, \
                 tc.tile_pool(name="psS", bufs=2, space="PSUM") as psS, \
                 tc.tile_pool(name="psPV", bufs=2, space="PSUM") as psPV, \
                 tc.tile_pool(name="pt", bufs=4) as ptp, \
                 tc.tile_pool(name="osb", bufs=2) as osbp, \
                 tc.tile_pool(name="rec", bufs=8) as recp, \
                 tc.tile_pool(name="od", bufs=4) as odp:
                def qproj_pieces(qb, do):
                    # two ~450ns PE pieces sharing one accumulation tile
                    hold = {}

                    def run_a():
                        hold["t"] = psB.tile([P, 512], F32, tag="big", name="qps")
                        for ko in range(4):
                            nc.tensor.matmul(
                                hold["t"][:],
                                wq_sb[:, ko, do * P:(do + 1) * P],
                                xT[:, ko, qb * 512:(qb + 1) * 512],
                                start=(ko == 0), stop=False,
                            )

                    def run_b():
                        for ko in range(4, KO):
                            nc.tensor.matmul(
                                hold["t"][:],
                                wq_sb[:, ko, do * P:(do + 1) * P],
                                xT[:, ko, qb * 512:(qb + 1) * 512],
                                start=False, stop=(ko == KO - 1),
                            )
                        nc.vector.tensor_copy(
                            qT[:, do, qb * 512:(qb + 1) * 512], hold["t"][:]
                        )
                    return [run_a, run_b]

                def transpose_pieces(q0, o_sb):
                    def run():
                        o_flat = o_sb[:].rearrange("p a b -> p (a b)")
                        otp = psB.tile([P, 512], F32, tag="big")
                        for d in range(DO):
                            nc.tensor.transpose(
                                otp[:, d * P:(d + 1) * P],
                                o_flat[:, d * P:(d + 1) * P],
                                ident[:],
                            )
                        nc.vector.tensor_copy(
                            oT_sb[:, :, q0:q0 + P],
                            otp[:].rearrange("p (a b) -> p a b", a=DO),
                        )
                    return [run]

                def proj_pieces(q0, ec):
                    hold = {}

                    def run_a():
                        hold["t"] = psB.tile([P, 512], F32, tag="big", name="fps")
                        for do in range(2):
                            nc.tensor.matmul(
                                hold["t"][:],
                                oT_sb[:, do, q0:q0 + P],
                                wo_sb[:, do, ec * 512:(ec + 1) * 512],
                                start=(do == 0), stop=False,
                            )

                    def run_b():
                        for do in range(2, DO):
                            nc.tensor.matmul(
                                hold["t"][:],
                                oT_sb[:, do, q0:q0 + P],
                                wo_sb[:, do, ec * 512:(ec + 1) * 512],
                                start=False, stop=(do == DO - 1),
                            )
                        ot = odp.tile([P, 512], F32, tag="otile")
                        nc.vector.tensor_tensor(
                            ot[:], hold["t"][:], bo_sb[:, ec * 512:(ec + 1) * 512],
                            mybir.AluOpType.add,
                        )
                        nc.sync.dma_start(
                            out_d[q0:q0 + P, ec * 512:(ec + 1) * 512], ot[:]
                        )
                    return [run_a, run_b]

                # ---------------- K.T projection ------------------------
                for do in range(DO):
                    for ms in range(2):
                        kps = psB.tile([P, 512], F32, tag="big")
                        for ko in range(KO):
                            nc.tensor.matmul(
                                kps[:],
                                wk_sb[:, ko, do * P:(do + 1) * P],
                                ctxT[:, ko, ms * 512:(ms + 1) * 512],
                                start=(ko == 0), stop=(ko == KO - 1),
                            )
                        nc.vector.tensor_copy(kT[:, do, ms * 512:(ms + 1) * 512], kps[:])
                # Q.T for block 0 (so exps can start before V is ready)
                for do in range(DO):
                    for piece in qproj_pieces(0, do):
                        piece()
                # ---------------- V projection --------------------------
                for mo in range(MT):
                    vps = psB.tile([P, 512], F32, tag="big")
                    for ko in range(KO):
                        nc.tensor.matmul(
                            vps[:],
                            ctxT[:, ko, mo * P:(mo + 1) * P],
                            wv_sb[:, ko, :],
                            start=(ko == 0), stop=(ko == KO - 1),
                        )
                    nc.vector.tensor_copy(
                        v_sb[:, mo, :, 0:DH],
                        vps[:].rearrange("p (h d) -> p h d", h=HL),
                    )

                # ------ attention: per q-tile, finalize work of the ------
                # ------ previous q-tile interleaved into the head loop ---
                pending = []
                for qt in range(NT):
                    q0 = qt * P
                    o_sb = osbp.tile([P, HL, DH], F32, tag="osb")
                    for h in range(HL):
                        pb = (h % 2) * DH
                        sps = psS.tile([P, MT, P], F32, tag="s")
                        for mo in range(MT):
                            nc.tensor.matmul(
                                sps[:, mo],
                                kT[pb:pb + DH, h // 2, mo * P:(mo + 1) * P],
                                qT[pb:pb + DH, h // 2, q0:q0 + P],
                                start=True, stop=True,
                                skip_group_check=True,
                            )
                        if pending:
                            pending.pop(0)()
                        if len(pending) >= 8:
                            pending.pop(0)()
                        ptile = ptp.tile([P, MT, P], BF16, tag="pt")
                        nc.scalar.activation(
                            ptile[:], sps[:],
                            mybir.ActivationFunctionType.Exp, scale=SCALE,
                        )
                        pv = psPV.tile([P, DH + 1], F32, tag="pv")
                        for mo in range(MT):
                            nc.tensor.matmul(
                                pv[:],
                                ptile[:, mo],
                                v_sb[:, mo, h, :],
                                start=(mo == 0), stop=(mo == MT - 1),
                                skip_group_check=True,
                            )
                        rec = recp.tile([P, 1], F32, tag="rec")
                        nc.vector.reciprocal(rec[:], pv[:, DH:DH + 1])
                        nc.vector.tensor_scalar_mul(o_sb[:, h, :], pv[:, 0:DH], rec[:])
                    pending.extend(transpose_pieces(q0, o_sb))
                    pending.extend(proj_pieces(q0, 0))
                    pending.extend(proj_pieces(q0, 1))
                    if qt % 4 == 1 and qt < 12:
                        # Q.T for the next 512-query block, ahead of its use
                        for do in range(DO):
                            pending.extend(qproj_pieces(qt // 4 + 1, do))
                for t in pending:
                    t()
    nc.finalize()
    return nc


def _get_nc():
    if "nc" not in _CACHE:
        _CACHE["nc"] = _build()
    return _CACHE["nc"]


def kernel(x, context, Wq, Wk, Wv, Wo, bo, **extra):
    nc = _get_nc()
    B = x.shape[0]
    bf = ml_dtypes.bfloat16
    ident = np.eye(P, dtype=np.float32)
    bo_b = np.broadcast_to(np.asarray(bo, dtype=np.float32), (P, E)).copy()
    zeros_bo = np.zeros((P, E), dtype=np.float32)
    x_b = np.asarray(x, dtype=bf)
    ctx_b = np.asarray(context, dtype=bf)
    wq_b = np.asarray(Wq, dtype=bf)
    wk_b = np.asarray(Wk, dtype=bf)
    wv_b = np.asarray(Wv, dtype=bf)
    wo_b = np.asarray(Wo, dtype=bf)
    in_maps = []
    for c in range(8):
        b, g = c // 2, c % 2
        in_maps.append({
            "x": np.ascontiguousarray(x_b[b]),
            "ctx": np.ascontiguousarray(ctx_b[b]),
            "wq": np.ascontiguousarray(wq_b[:, g * DHG:(g + 1) * DHG]),
            "wk": np.ascontiguousarray(wk_b[:, g * DHG:(g + 1) * DHG]),
            "wv": np.ascontiguousarray(wv_b[:, g * DHG:(g + 1) * DHG]),
            "wo": np.ascontiguousarray(wo_b[g * DHG:(g + 1) * DHG, :]),
            "bo": (bo_b if g == 0 else zeros_bo),
            "ident": ident,
        })
    global _last_in_maps
    _last_in_maps = in_maps
    res = run_bass_kernel_spmd(nc, in_maps, list(range(8)))
    out = np.empty((B, N, E), dtype=np.float32)
    for b in range(B):
        out[b] = res.results[2 * b]["out"] + res.results[2 * b + 1]["out"]
    return out


# revision 27
# speedup vs baseline: 1.0285x; 1.0285x over previous
"""Cross-attention Trainium2 kernel (8 NeuronCores, SPMD).

Sharding: core c handles batch c//2 and head-group c%2 (8 of 16 heads).
Each core computes its head-group's partial output projection; the host
sums the two partials per batch (bias is folded into head-group 0).

Shapes (hardcoded): B=4, N=2048 (queries), M=1024 (context), K=1024
(query/context dim), H=16 heads, DH=64, head-group width DHG=512, E=1024.

Dataflow (bf16 compute, fp32 PSUM accumulation / fp32 output):
  host pre-casts x/ctx/weights to bf16 and pre-broadcasts bias.
  xT/ctxT loaded via XBAR DMA transpose (no PE transposes for inputs).
  K.T = Wk.T @ ctxT, V = ctxT.T @ Wv, Q.T = Wq.T @ xT (bf16 matmuls).
  Per (q-tile of 128, head): S.T[m,q] (8 matmuls, d=64), P.T = exp on ACT
  (one 1024-wide activation per head), PV in [q-part, 65] orientation
  (V plus ones column -> softmax row sums land in column 64), reciprocal +
  per-partition-scalar normalize on DVE, O per q-tile PE-transposed back
  to O.T, out = O.T.T @ Wo; bias added by DVE during the PSUM->SBUF copy.
  The finalize work of each q-tile (transpose, output projection, store)
  and the next block's Q.T projection are split into ~450ns pieces and
  interleaved one-per-head into the following q-tile's S/exp/PV loop so
  PE stays busy during the exp latency of the S->exp->PV chain.
"""
import sys

if "/opt/trn_rl_repo" not in sys.path:
    sys.path.insert(0, "/opt/trn_rl_repo")

import numpy as np
import ml_dtypes

import concourse.bass as bass  # noqa: F401
import concourse.tile as tile
from concourse import bacc, mybir
from concourse.bass_utils import run_bass_kernel_spmd

P = 128
N = 2048          # queries per batch
M = 1024          # context rows
K = 1024          # query_dim == context_dim
DHG = 512         # d_attn per head group (8 heads x 64)
DH = 64           # dim per head
HL = 8            # heads per core
E = 1024          # output dim
SCALE = DH ** -0.5
F32 = mybir.dt.float32
BF16 = mybir.dt.bfloat16

KO = K // P       # 8 contraction chunks
NT = N // P       # 16 query tiles
MT = M // P       # 8 context tiles
DO = DHG // P     # 4 head-dim chunks
QC = N // 512     # 4 query blocks of 512
EC = E // 512     # 2 output chunks of 512

_CACHE = {}


def _build():
    nc = bacc.Bacc("TRN2", target_bir_lowering=False, debug=False, num_devices=8)
    x_d = nc.dram_tensor("x", [N, K], BF16, kind="ExternalInput")
    ctx_d = nc.dram_tensor("ctx", [M, K], BF16, kind="ExternalInput")
    wq_d = nc.dram_tensor("wq", [K, DHG], BF16, kind="ExternalInput")
    wk_d = nc.dram_tensor("wk", [K, DHG], BF16, kind="ExternalInput")
    wv_d = nc.dram_tensor("wv", [K, DHG], BF16, kind="ExternalInput")
    wo_d = nc.dram_tensor("wo", [DHG, E], BF16, kind="ExternalInput")
    bo_d = nc.dram_tensor("bo", [P, E], F32, kind="ExternalInput")
    id_d = nc.dram_tensor("ident", [P, P], F32, kind="ExternalInput")
    out_d = nc.dram_tensor("out", [N, E], F32, kind="ExternalOutput")

    with tile.TileContext(nc) as tc:
        with tc.tile_pool(name="persist", bufs=1) as pp:
            # DMA issue order tuned so K-proj (wk+ctxT) then Q-proj (wq+xT0)
            # inputs arrive first on the serialized DMA device.
            ident = pp.tile([P, P], F32)
            bo_sb = pp.tile([P, E], F32)
            wq_sb = pp.tile([P, KO, DHG], BF16)
            wk_sb = pp.tile([P, KO, DHG], BF16)
            wv_sb = pp.tile([P, KO, DHG], BF16)
            wo_sb = pp.tile([P, DO, E], BF16)
            ctxT = pp.tile([P, KO, M], BF16)
            xT = pp.tile([P, KO, N], BF16)

            nc.sync.dma_start(wk_sb[:], wk_d.rearrange("(ko p) d -> p ko d", p=P))
            for ms in range(2):
                nc.sync.dma_start_transpose(
                    ctxT[:, :, ms * 512:(ms + 1) * 512],
                    ctx_d[ms * 512:(ms + 1) * 512, :],
                )
            nc.sync.dma_start(wq_sb[:], wq_d.rearrange("(ko p) d -> p ko d", p=P))
            nc.sync.dma_start_transpose(xT[:, :, 0:512], x_d[0:512, :])
            nc.sync.dma_start(wv_sb[:], wv_d.rearrange("(ko p) d -> p ko d", p=P))
            nc.sync.dma_start(wo_sb[:], wo_d.rearrange("(do p) e -> p do e", p=P))
            for ns in range(1, QC):
                nc.sync.dma_start_transpose(
                    xT[:, :, ns * 512:(ns + 1) * 512],
                    x_d[ns * 512:(ns + 1) * 512, :],
                )
            nc.sync.dma_start(ident[:], id_d[:])
            nc.sync.dma_start(bo_sb[:], bo_d[:])

            kT = pp.tile([P, DO, M], BF16)        # K.T  [dhg, m]
            qT = pp.tile([P, DO, N], BF16)        # Q.T  [dhg, n]
            v_sb = pp.tile([P, MT, HL, DH + 1], BF16)  # V + ones col per head
            oT_sb = pp.tile([P, DO, N], BF16)     # O.T  [dhg, n] normalized
            nc.vector.memset(v_sb[:, :, :, DH], 1.0)

            with tc.tile_pool(name="psBig", bufs=2, space="PSUM") as psB, \
                 tc.tile_pool(name="psS", bufs=2, space="PSUM") as psS, \
                 tc.tile_pool(name="psPV", bufs=2, space="PSUM") as psPV, \
                 tc.tile_pool(name="pt", bufs=4) as ptp, \
                 tc.tile_pool(name="osb", bufs=2) as osbp, \
                 tc.tile_pool(name="rec", bufs=8) as recp, \
                 tc.tile_pool(name="od", bufs=4) as odp:
                def qproj_pieces(qb, do):
                    # two ~450ns PE pieces sharing one accumulation tile
                    hold = {}

                    def run_a():
                        hold["t"] = psB.tile([P, 512], F32, tag="big", name="qps")
                        for ko in range(4):
                            nc.tensor.matmul(
                                hold["t"][:],
                                wq_sb[:, ko, do * P:(do + 1) * P],
                                xT[:, ko, qb * 512:(qb + 1) * 512],
                                start=(ko == 0), stop=False,
                            )

                    def run_b():
                        for ko in range(4, KO):
                            nc.tensor.matmul(
                                hold["t"][:],
                                wq_sb[:, ko, do * P:(do + 1) * P],
                                xT[:, ko, qb * 512:(qb + 1) * 512],
                                start=False, stop=(ko == KO - 1),
                            )
                        nc.vector.tensor_copy(
                            qT[:, do, qb * 512:(qb + 1) * 512], hold["t"][:]
                        )
                    return [run_a, run_b]

                def transpose_pieces(q0, o_sb):
                    def run():
                        o_flat = o_sb[:].rearrange("p a b -> p (a b)")
                        otp = psB.tile([P, 512], F32, tag="big")
                        for d in range(DO):
                            nc.tensor.transpose(
                                otp[:, d * P:(d + 1) * P],
                                o_flat[:, d * P:(d + 1) * P],
                                ident[:],
                            )
                        nc.vector.tensor_copy(
                            oT_sb[:, :, q0:q0 + P],
                            otp[:].rearrange("p (a b) -> p a b", a=DO),
                        )
                    return [run]

                def proj_pieces(q0, ec):
                    hold = {}

                    def run_a():
                        hold["t"] = psB.tile([P, 512], F32, tag="big", name="fps")
                        for do in range(2):
                            nc.tensor.matmul(
                                hold["t"][:],
                                oT_sb[:, do, q0:q0 + P],
                                wo_sb[:, do, ec * 512:(ec + 1) * 512],
                                start=(do == 0), stop=False,
                            )

                    def run_b():
                        for do in range(2, DO):
                            nc.tensor.matmul(
                                hold["t"][:],
                                oT_sb[:, do, q0:q0 + P],
                                wo_sb[:, do, ec * 512:(ec + 1) * 512],
                                start=False, stop=(do == DO - 1),
                            )
                        ot = odp.tile([P, 512], F32, tag="otile")
                        nc.vector.tensor_tensor(
                            ot[:], hold["t"][:], bo_sb[:, ec * 512:(ec + 1) * 512],
                            mybir.AluOpType.add,
                        )
                        nc.sync.dma_start(
                            out_d[q0:q0 + P, ec * 512:(ec + 1) * 512], ot[:]
                        )
                    return [run_a, run_b]

                # ---------------- K.T projection ------------------------
                for do in range(DO):
                    for ms in range(2):
                        kps = psB.tile([P, 512], F32, tag="big")
                        for ko in range(KO):
                            nc.tensor.matmul(
                                kps[:],
                                wk_sb[:, ko, do * P:(do + 1) * P],
                                ctxT[:, ko, ms * 512:(ms + 1) * 512],
                                start=(ko == 0), stop=(ko == KO - 1),
                            )
                        nc.vector.tensor_copy(kT[:, do, ms * 512:(ms + 1) * 512], kps[:])
                # Q.T for block 0 (so exps can start before V is ready)
                for do in range(DO):
                    for piece in qproj_pieces(0, do):
                        piece()
                # ---------------- V projection --------------------------
                for mo in range(MT):
                    vps = psB.tile([P, 512], F32, tag="big")
                    for ko in range(KO):
                        nc.tensor.matmul(
                            vps[:],
                            ctxT[:, ko, mo * P:(mo + 1) * P],
                            wv_sb[:, ko, :],
                            start=(ko == 0), stop=(ko == KO - 1),
                        )
                    nc.vector.tensor_copy(
                        v_sb[:, mo, :, 0:DH],
                        vps[:].rearrange("p (h d) -> p h d", h=HL),
                    )

                # ------ attention: per q-tile, finalize work of the ------
                # ------ previous q-tile interleaved into the head loop ---
                pending = []
                for qt in range(NT):
                    q0 = qt * P
                    o_sb = osbp.tile([P, HL, DH], F32, tag="osb")
                    for h in range(HL):
                        pb = (h % 2) * DH
                        sps = psS.tile([P, MT, P], F32, tag="s")
                        for mo in range(MT):
                            nc.tensor.matmul(
                                sps[:, mo],
                                kT[pb:pb + DH, h // 2, mo * P:(mo + 1) * P],
                                qT[pb:pb + DH, h // 2, q0:q0 + P],
                                start=True, stop=True,
                                skip_group_check=True,
                            )
                        if pending and h > 0:
                            pending.pop(0)()
                        if len(pending) >= 8:
                            pending.pop(0)()
                        ptile = ptp.tile([P, MT, P], BF16, tag="pt")
                        nc.scalar.activation(
                            ptile[:], sps[:],
                            mybir.ActivationFunctionType.Exp, scale=SCALE,
                        )
                        pv = psPV.tile([P, DH + 1], F32, tag="pv")
                        for mo in range(MT):
                            nc.tensor.matmul(
                                pv[:],
                                ptile[:, mo],
                                v_sb[:, mo, h, :],
                                start=(mo == 0), stop=(mo == MT - 1),
                                skip_group_check=True,
                            )
                        rec = recp.tile([P, 1], F32, tag="rec")
                        nc.vector.reciprocal(rec[:], pv[:, DH:DH + 1])
                        nc.vector.tensor_scalar_mul(o_sb[:, h, :], pv[:, 0:DH], rec[:])
                    pending.extend(transpose_pieces(q0, o_sb))
                    pending.extend(proj_pieces(q0, 0))
                    pending.extend(proj_pieces(q0, 1))
                    if qt % 4 == 1 and qt < 12:
                        # Q.T for the next 512-query block, ahead of its use
                        for do in range(DO):
                            pending.extend(qproj_pieces(qt // 4 + 1, do))
                for t in pending:
                    t()
    nc.finalize()
    return nc


def _get_nc():
    if "nc" not in _CACHE:
        _CACHE["nc"] = _build()
    return _CACHE["nc"]


def kernel(x, context, Wq, Wk, Wv, Wo, bo, **extra):
    nc = _get_nc()
    B = x.shape[0]
    bf = ml_dtypes.bfloat16
    ident = np.eye(P, dtype=np.float32)
    bo_b = np.broadcast_to(np.asarray(bo, dtype=np.float32), (P, E)).copy()
    zeros_bo = np.zeros((P, E), dtype=np.float32)
    x_b = np.asarray(x, dtype=bf)
    ctx_b = np.asarray(context, dtype=bf)
    wq_b = np.asarray(Wq, dtype=bf)
    wk_b = np.asarray(Wk, dtype=bf)
    wv_b = np.asarray(Wv, dtype=bf)
    wo_b = np.asarray(Wo, dtype=bf)
    in_maps = []
    for c in range(8):
        b, g = c // 2, c % 2
        in_maps.append({
            "x": np.ascontiguousarray(x_b[b]),
            "ctx": np.ascontiguousarray(ctx_b[b]),
            "wq": np.ascontiguousarray(wq_b[:, g * DHG:(g + 1) * DHG]),
            "wk": np.ascontiguousarray(wk_b[:, g * DHG:(g + 1) * DHG]),
            "wv": np.ascontiguousarray(wv_b[:, g * DHG:(g + 1) * DHG]),
            "wo": np.ascontiguousarray(wo_b[g * DHG:(g + 1) * DHG, :]),
            "bo": (bo_b if g == 0 else zeros_bo),
            "ident": ident,
        })
    global _last_in_maps
    _last_in_maps = in_maps
    res = run_bass_kernel_spmd(nc, in_maps, list(range(8)))
    out = np.empty((B, N, E), dtype=np.float32)
    for b in range(B):
        out[b] = res.results[2 * b]["out"] + res.results[2 * b + 1]["out"]
    return out


# revision 28
# speedup vs baseline: 1.0316x; 1.0029x over previous
"""Cross-attention Trainium2 kernel (8 NeuronCores, SPMD).

Sharding: core c handles batch c//2 and head-group c%2 (8 of 16 heads).
Each core computes its head-group's partial output projection; the host
sums the two partials per batch (bias is folded into head-group 0).

Shapes (hardcoded): B=4, N=2048 (queries), M=1024 (context), K=1024
(query/context dim), H=16 heads, DH=64, head-group width DHG=512, E=1024.

Dataflow (bf16 compute, fp32 PSUM accumulation / fp32 output):
  host pre-casts x/ctx/weights to bf16 and pre-broadcasts bias.
  xT/ctxT loaded via XBAR DMA transpose (no PE transposes for inputs).
  K.T = Wk.T @ ctxT, V = ctxT.T @ Wv, Q.T = Wq.T @ xT (bf16 matmuls).
  Per (q-tile of 128, head): S.T[m,q] (8 matmuls, d=64), P.T = exp on ACT
  (one 1024-wide activation per head), PV in [q-part, 65] orientation
  (V plus ones column -> softmax row sums land in column 64), reciprocal +
  per-partition-scalar normalize on DVE, O per q-tile PE-transposed back
  to O.T, out = O.T.T @ Wo; bias added by DVE during the PSUM->SBUF copy.
  The finalize work of each q-tile (transpose, output projection, store)
  and the next block's Q.T projection are split into ~450ns pieces and
  interleaved one-per-head into the following q-tile's S/exp/PV loop so
  PE stays busy during the exp latency of the S->exp->PV chain.
"""
import sys

if "/opt/trn_rl_repo" not in sys.path:
    sys.path.insert(0, "/opt/trn_rl_repo")

import numpy as np
import ml_dtypes

import concourse.bass as bass  # noqa: F401
import concourse.tile as tile
from concourse import bacc, mybir
from concourse.bass_utils import run_bass_kernel_spmd

P = 128
N = 2048          # queries per batch
M = 1024          # context rows
K = 1024          # query_dim == context_dim
DHG = 512         # d_attn per head group (8 heads x 64)
DH = 64           # dim per head
HL = 8            # heads per core
E = 1024          # output dim
SCALE = DH ** -0.5
F32 = mybir.dt.float32
BF16 = mybir.dt.bfloat16

KO = K // P       # 8 contraction chunks
NT = N // P       # 16 query tiles
MT = M // P       # 8 context tiles
DO = DHG // P     # 4 head-dim chunks
QC = N // 512     # 4 query blocks of 512
EC = E // 512     # 2 output chunks of 512

_CACHE = {}


def _build():
    nc = bacc.Bacc("TRN2", target_bir_lowering=False, debug=False, num_devices=8)
    x_d = nc.dram_tensor("x", [N, K], BF16, kind="ExternalInput")
    ctx_d = nc.dram_tensor("ctx", [M, K], BF16, kind="ExternalInput")
    wq_d = nc.dram_tensor("wq", [K, DHG], BF16, kind="ExternalInput")
    wk_d = nc.dram_tensor("wk", [K, DHG], BF16, kind="ExternalInput")
    wv_d = nc.dram_tensor("wv", [K, DHG], BF16, kind="ExternalInput")
    wo_d = nc.dram_tensor("wo", [DHG, E], BF16, kind="ExternalInput")
    bo_d = nc.dram_tensor("bo", [P, E], F32, kind="ExternalInput")
    id_d = nc.dram_tensor("ident", [P, P], F32, kind="ExternalInput")
    out_d = nc.dram_tensor("out", [N, E], F32, kind="ExternalOutput")

    with tile.TileContext(nc) as tc:
        with tc.tile_pool(name="persist", bufs=1) as pp:
            # DMA issue order tuned so K-proj (wk+ctxT) then Q-proj (wq+xT0)
            # inputs arrive first on the serialized DMA device.
            ident = pp.tile([P, P], F32)
            bo_sb = pp.tile([P, E], F32)
            wq_sb = pp.tile([P, KO, DHG], BF16)
            wk_sb = pp.tile([P, KO, DHG], BF16)
            wv_sb = pp.tile([P, KO, DHG], BF16)
            wo_sb = pp.tile([P, DO, E], BF16)
            ctxT = pp.tile([P, KO, M], BF16)
            xT = pp.tile([P, KO, N], BF16)

            nc.sync.dma_start(wk_sb[:], wk_d.rearrange("(ko p) d -> p ko d", p=P))
            for ms in range(2):
                nc.sync.dma_start_transpose(
                    ctxT[:, :, ms * 512:(ms + 1) * 512],
                    ctx_d[ms * 512:(ms + 1) * 512, :],
                )
            nc.sync.dma_start(wq_sb[:], wq_d.rearrange("(ko p) d -> p ko d", p=P))
            nc.sync.dma_start_transpose(xT[:, :, 0:512], x_d[0:512, :])
            nc.sync.dma_start(wv_sb[:], wv_d.rearrange("(ko p) d -> p ko d", p=P))
            nc.sync.dma_start(wo_sb[:], wo_d.rearrange("(do p) e -> p do e", p=P))
            for ns in range(1, QC):
                nc.sync.dma_start_transpose(
                    xT[:, :, ns * 512:(ns + 1) * 512],
                    x_d[ns * 512:(ns + 1) * 512, :],
                )
            nc.sync.dma_start(ident[:], id_d[:])
            nc.sync.dma_start(bo_sb[:], bo_d[:])

            kT = pp.tile([P, DO, M], BF16)        # K.T  [dhg, m]
            qT = pp.tile([P, DO, N], BF16)        # Q.T  [dhg, n]
            v_sb = pp.tile([P, MT, HL, DH + 1], BF16)  # V + ones col per head
            oT_sb = pp.tile([P, DO, N], BF16)     # O.T  [dhg, n] normalized
            nc.vector.memset(v_sb[:, :, :, DH], 1.0)

            with tc.tile_pool(name="psBig", bufs=2, space="PSUM") as psB, \
                 tc.tile_pool(name="psS", bufs=2, space="PSUM") as psS, \
                 tc.tile_pool(name="psPV", bufs=2, space="PSUM") as psPV, \
                 tc.tile_pool(name="pt", bufs=4) as ptp, \
                 tc.tile_pool(name="osb", bufs=2) as osbp, \
                 tc.tile_pool(name="rec", bufs=8) as recp, \
                 tc.tile_pool(name="od", bufs=4) as odp:
                def qproj_pieces(qb, do):
                    # two ~450ns PE pieces sharing one accumulation tile
                    hold = {}

                    def run_a():
                        hold["t"] = psB.tile([P, 512], F32, tag="big", name="qps")
                        for ko in range(4):
                            nc.tensor.matmul(
                                hold["t"][:],
                                wq_sb[:, ko, do * P:(do + 1) * P],
                                xT[:, ko, qb * 512:(qb + 1) * 512],
                                start=(ko == 0), stop=False,
                            )

                    def run_b():
                        for ko in range(4, KO):
                            nc.tensor.matmul(
                                hold["t"][:],
                                wq_sb[:, ko, do * P:(do + 1) * P],
                                xT[:, ko, qb * 512:(qb + 1) * 512],
                                start=False, stop=(ko == KO - 1),
                            )
                        nc.vector.tensor_copy(
                            qT[:, do, qb * 512:(qb + 1) * 512], hold["t"][:]
                        )
                    return [run_a, run_b]

                def transpose_pieces(q0, o_sb):
                    def run():
                        o_flat = o_sb[:].rearrange("p a b -> p (a b)")
                        otp = psB.tile([P, 512], F32, tag="big")
                        for d in range(DO):
                            nc.tensor.transpose(
                                otp[:, d * P:(d + 1) * P],
                                o_flat[:, d * P:(d + 1) * P],
                                ident[:],
                            )
                        nc.vector.tensor_copy(
                            oT_sb[:, :, q0:q0 + P],
                            otp[:].rearrange("p (a b) -> p a b", a=DO),
                        )
                    return [run]

                def proj_pieces(q0, ec):
                    hold = {}

                    def run_a():
                        hold["t"] = psB.tile([P, 512], F32, tag="big", name="fps")
                        for do in range(2):
                            nc.tensor.matmul(
                                hold["t"][:],
                                oT_sb[:, do, q0:q0 + P],
                                wo_sb[:, do, ec * 512:(ec + 1) * 512],
                                start=(do == 0), stop=False,
                            )

                    def run_b():
                        for do in range(2, DO):
                            nc.tensor.matmul(
                                hold["t"][:],
                                oT_sb[:, do, q0:q0 + P],
                                wo_sb[:, do, ec * 512:(ec + 1) * 512],
                                start=False, stop=(do == DO - 1),
                            )
                        ot = odp.tile([P, 512], F32, tag="otile")
                        nc.vector.tensor_tensor(
                            ot[:], hold["t"][:], bo_sb[:, ec * 512:(ec + 1) * 512],
                            mybir.AluOpType.add,
                        )
                        nc.sync.dma_start(
                            out_d[q0:q0 + P, ec * 512:(ec + 1) * 512], ot[:]
                        )
                    return [run_a, run_b]

                # ---------------- K.T projection ------------------------
                for do in range(DO):
                    for ms in range(2):
                        kps = psB.tile([P, 512], F32, tag="big")
                        for ko in range(KO):
                            nc.tensor.matmul(
                                kps[:],
                                wk_sb[:, ko, do * P:(do + 1) * P],
                                ctxT[:, ko, ms * 512:(ms + 1) * 512],
                                start=(ko == 0), stop=(ko == KO - 1),
                            )
                        nc.vector.tensor_copy(kT[:, do, ms * 512:(ms + 1) * 512], kps[:])
                # Q.T for block 0 (so exps can start before V is ready)
                for do in range(DO):
                    for piece in qproj_pieces(0, do):
                        piece()
                # ---------------- V projection --------------------------
                for mo in range(MT):
                    vps = psB.tile([P, 512], F32, tag="big")
                    for ko in range(KO):
                        nc.tensor.matmul(
                            vps[:],
                            ctxT[:, ko, mo * P:(mo + 1) * P],
                            wv_sb[:, ko, :],
                            start=(ko == 0), stop=(ko == KO - 1),
                        )
                    nc.vector.tensor_copy(
                        v_sb[:, mo, :, 0:DH],
                        vps[:].rearrange("p (h d) -> p h d", h=HL),
                    )

                # ------ attention: per q-tile, finalize work of the ------
                # ------ previous q-tile interleaved into the head loop ---
                pending = []
                for qt in range(NT):
                    q0 = qt * P
                    o_sb = osbp.tile([P, HL, DH], F32, tag="osb")
                    for h in range(HL):
                        pb = (h % 2) * DH
                        sps = psS.tile([P, MT, P], F32, tag="s")
                        for mo in range(MT):
                            nc.tensor.matmul(
                                sps[:, mo],
                                kT[pb:pb + DH, h // 2, mo * P:(mo + 1) * P],
                                qT[pb:pb + DH, h // 2, q0:q0 + P],
                                start=True, stop=True,
                                skip_group_check=True,
                            )
                        if pending and h > 1:
                            pending.pop(0)()
                        if len(pending) >= 8:
                            pending.pop(0)()
                        ptile = ptp.tile([P, MT, P], BF16, tag="pt")
                        nc.scalar.activation(
                            ptile[:], sps[:],
                            mybir.ActivationFunctionType.Exp, scale=SCALE,
                        )
                        pv = psPV.tile([P, DH + 1], F32, tag="pv")
                        for mo in range(MT):
                            nc.tensor.matmul(
                                pv[:],
                                ptile[:, mo],
                                v_sb[:, mo, h, :],
                                start=(mo == 0), stop=(mo == MT - 1),
                                skip_group_check=True,
                            )
                        rec = recp.tile([P, 1], F32, tag="rec")
                        nc.vector.reciprocal(rec[:], pv[:, DH:DH + 1])
                        nc.vector.tensor_scalar_mul(o_sb[:, h, :], pv[:, 0:DH], rec[:])
                    pending.extend(transpose_pieces(q0, o_sb))
                    pending.extend(proj_pieces(q0, 0))
                    pending.extend(proj_pieces(q0, 1))
                    if qt % 4 == 1 and qt < 12:
                        # Q.T for the next 512-query block, ahead of its use
                        for do in range(DO):
                            pending.extend(qproj_pieces(qt // 4 + 1, do))
                for t in pending:
                    t()
    nc.finalize()
    return nc


def _get_nc():
    if "nc" not in _CACHE:
        _CACHE["nc"] = _build()
    return _CACHE["nc"]


def kernel(x, context, Wq, Wk, Wv, Wo, bo, **extra):
    nc = _get_nc()
    B = x.shape[0]
    bf = ml_dtypes.bfloat16
    ident = np.eye(P, dtype=np.float32)
    bo_b = np.broadcast_to(np.asarray(bo, dtype=np.float32), (P, E)).copy()
    zeros_bo = np.zeros((P, E), dtype=np.float32)
    x_b = np.asarray(x, dtype=bf)
    ctx_b = np.asarray(context, dtype=bf)
    wq_b = np.asarray(Wq, dtype=bf)
    wk_b = np.asarray(Wk, dtype=bf)
    wv_b = np.asarray(Wv, dtype=bf)
    wo_b = np.asarray(Wo, dtype=bf)
    in_maps = []
    for c in range(8):
        b, g = c // 2, c % 2
        in_maps.append({
            "x": np.ascontiguousarray(x_b[b]),
            "ctx": np.ascontiguousarray(ctx_b[b]),
            "wq": np.ascontiguousarray(wq_b[:, g * DHG:(g + 1) * DHG]),
            "wk": np.ascontiguousarray(wk_b[:, g * DHG:(g + 1) * DHG]),
            "wv": np.ascontiguousarray(wv_b[:, g * DHG:(g + 1) * DHG]),
            "wo": np.ascontiguousarray(wo_b[g * DHG:(g + 1) * DHG, :]),
            "bo": (bo_b if g == 0 else zeros_bo),
            "ident": ident,
        })
    global _last_in_maps
    _last_in_maps = in_maps
    res = run_bass_kernel_spmd(nc, in_maps, list(range(8)))
    out = np.empty((B, N, E), dtype=np.float32)
    for b in range(B):
        out[b] = res.results[2 * b]["out"] + res.results[2 * b + 1]["out"]
    return out


# revision 29
# speedup vs baseline: 1.0413x; 1.0095x over previous
"""Cross-attention Trainium2 kernel (8 NeuronCores, SPMD).

Sharding: core c handles batch c//2 and head-group c%2 (8 of 16 heads).
Each core computes its head-group's partial output projection; the host
sums the two partials per batch (bias is folded into head-group 0).

Shapes (hardcoded): B=4, N=2048 (queries), M=1024 (context), K=1024
(query/context dim), H=16 heads, DH=64, head-group width DHG=512, E=1024.

Dataflow (bf16 compute, fp32 PSUM accumulation / fp32 output):
  host pre-casts x/ctx/weights to bf16 and pre-broadcasts bias.
  xT/ctxT loaded via XBAR DMA transpose (no PE transposes for inputs).
  K.T = Wk.T @ ctxT, V = ctxT.T @ Wv, Q.T = Wq.T @ xT (bf16 matmuls).
  Per (q-tile of 128, head): S.T[m,q] (8 matmuls, d=64), P.T = exp on ACT
  (one 1024-wide activation per head), PV in [q-part, 65] orientation
  (V plus ones column -> softmax row sums land in column 64), reciprocal +
  per-partition-scalar normalize on DVE, O per q-tile PE-transposed back
  to O.T, out = O.T.T @ Wo; bias added by DVE during the PSUM->SBUF copy.
  The finalize work of each q-tile (transpose, output projection, store)
  and the next block's Q.T projection are split into ~450ns pieces and
  interleaved one-per-head into the following q-tile's S/exp/PV loop so
  PE stays busy during the exp latency of the S->exp->PV chain.
"""
import sys

if "/opt/trn_rl_repo" not in sys.path:
    sys.path.insert(0, "/opt/trn_rl_repo")

import numpy as np
import ml_dtypes

import concourse.bass as bass  # noqa: F401
import concourse.tile as tile
from concourse import bacc, mybir
from concourse.bass_utils import run_bass_kernel_spmd

P = 128
N = 2048          # queries per batch
M = 1024          # context rows
K = 1024          # query_dim == context_dim
DHG = 512         # d_attn per head group (8 heads x 64)
DH = 64           # dim per head
HL = 8            # heads per core
E = 1024          # output dim
SCALE = DH ** -0.5
F32 = mybir.dt.float32
BF16 = mybir.dt.bfloat16

KO = K // P       # 8 contraction chunks
NT = N // P       # 16 query tiles
MT = M // P       # 8 context tiles
DO = DHG // P     # 4 head-dim chunks
QC = N // 512     # 4 query blocks of 512
EC = E // 512     # 2 output chunks of 512

_CACHE = {}


def _build():
    nc = bacc.Bacc("TRN2", target_bir_lowering=False, debug=False, num_devices=8)
    x_d = nc.dram_tensor("x", [N, K], BF16, kind="ExternalInput")
    ctx_d = nc.dram_tensor("ctx", [M, K], BF16, kind="ExternalInput")
    wq_d = nc.dram_tensor("wq", [K, DHG], BF16, kind="ExternalInput")
    wk_d = nc.dram_tensor("wk", [K, DHG], BF16, kind="ExternalInput")
    wv_d = nc.dram_tensor("wv", [K, DHG], BF16, kind="ExternalInput")
    wo_d = nc.dram_tensor("wo", [DHG, E], BF16, kind="ExternalInput")
    bo_d = nc.dram_tensor("bo", [P, E], F32, kind="ExternalInput")
    id_d = nc.dram_tensor("ident", [P, P], F32, kind="ExternalInput")
    out_d = nc.dram_tensor("out", [N, E], F32, kind="ExternalOutput")

    with tile.TileContext(nc) as tc:
        with tc.tile_pool(name="persist", bufs=1) as pp:
            # DMA issue order tuned so K-proj (wk+ctxT) then Q-proj (wq+xT0)
            # inputs arrive first on the serialized DMA device.
            ident = pp.tile([P, P], F32)
            bo_sb = pp.tile([P, E], F32)
            wq_sb = pp.tile([P, KO, DHG], BF16)
            wk_sb = pp.tile([P, KO, DHG], BF16)
            wv_sb = pp.tile([P, KO, DHG], BF16)
            wo_sb = pp.tile([P, DO, E], BF16)
            ctxT = pp.tile([P, KO, M], BF16)
            xT = pp.tile([P, KO, N], BF16)

            nc.sync.dma_start(wk_sb[:], wk_d.rearrange("(ko p) d -> p ko d", p=P))
            for ms in range(2):
                nc.sync.dma_start_transpose(
                    ctxT[:, :, ms * 512:(ms + 1) * 512],
                    ctx_d[ms * 512:(ms + 1) * 512, :],
                )
            nc.sync.dma_start(wq_sb[:], wq_d.rearrange("(ko p) d -> p ko d", p=P))
            nc.sync.dma_start_transpose(xT[:, :, 0:512], x_d[0:512, :])
            nc.sync.dma_start(wv_sb[:], wv_d.rearrange("(ko p) d -> p ko d", p=P))
            nc.sync.dma_start(wo_sb[:], wo_d.rearrange("(do p) e -> p do e", p=P))
            for ns in range(1, QC):
                nc.sync.dma_start_transpose(
                    xT[:, :, ns * 512:(ns + 1) * 512],
                    x_d[ns * 512:(ns + 1) * 512, :],
                )
            nc.sync.dma_start(ident[:], id_d[:])
            nc.sync.dma_start(bo_sb[:], bo_d[:])

            kT = pp.tile([P, DO, M], BF16)        # K.T  [dhg, m]
            qT = pp.tile([P, DO, N], BF16)        # Q.T  [dhg, n]
            v_sb = pp.tile([P, MT, HL, DH + 1], BF16)  # V + ones col per head
            oT_sb = pp.tile([P, DO, N], BF16)     # O.T  [dhg, n] normalized
            nc.vector.memset(v_sb[:, :, :, DH], 1.0)

            with tc.tile_pool(name="psBig", bufs=2, space="PSUM") as psB, \
                 tc.tile_pool(name="psS", bufs=2, space="PSUM") as psS, \
                 tc.tile_pool(name="psPV", bufs=2, space="PSUM") as psPV, \
                 tc.tile_pool(name="pt", bufs=4) as ptp, \
                 tc.tile_pool(name="osb", bufs=2) as osbp, \
                 tc.tile_pool(name="rec", bufs=8) as recp, \
                 tc.tile_pool(name="od", bufs=4) as odp:
                def qproj_pieces(qb, do):
                    # two ~450ns PE pieces sharing one accumulation tile
                    hold = {}

                    def run_a():
                        hold["t"] = psB.tile([P, 512], F32, tag="big", name="qps")
                        for ko in range(4):
                            nc.tensor.matmul(
                                hold["t"][:],
                                wq_sb[:, ko, do * P:(do + 1) * P],
                                xT[:, ko, qb * 512:(qb + 1) * 512],
                                start=(ko == 0), stop=False,
                            )

                    def run_b():
                        for ko in range(4, KO):
                            nc.tensor.matmul(
                                hold["t"][:],
                                wq_sb[:, ko, do * P:(do + 1) * P],
                                xT[:, ko, qb * 512:(qb + 1) * 512],
                                start=False, stop=(ko == KO - 1),
                            )
                        nc.vector.tensor_copy(
                            qT[:, do, qb * 512:(qb + 1) * 512], hold["t"][:]
                        )
                    return [run_a, run_b]

                def transpose_pieces(q0, o_sb):
                    def run():
                        o_flat = o_sb[:].rearrange("p a b -> p (a b)")
                        otp = psB.tile([P, 512], F32, tag="big")
                        for d in range(DO):
                            nc.tensor.transpose(
                                otp[:, d * P:(d + 1) * P],
                                o_flat[:, d * P:(d + 1) * P],
                                ident[:],
                            )
                        nc.vector.tensor_copy(
                            oT_sb[:, :, q0:q0 + P],
                            otp[:].rearrange("p (a b) -> p a b", a=DO),
                        )
                    return [run]

                def proj_pieces(q0, ec):
                    hold = {}

                    def run_a():
                        hold["t"] = psB.tile([P, 512], F32, tag="big", name="fps")
                        for do in range(2):
                            nc.tensor.matmul(
                                hold["t"][:],
                                oT_sb[:, do, q0:q0 + P],
                                wo_sb[:, do, ec * 512:(ec + 1) * 512],
                                start=(do == 0), stop=False,
                            )

                    def run_b():
                        for do in range(2, DO):
                            nc.tensor.matmul(
                                hold["t"][:],
                                oT_sb[:, do, q0:q0 + P],
                                wo_sb[:, do, ec * 512:(ec + 1) * 512],
                                start=False, stop=(do == DO - 1),
                            )
                        ot = odp.tile([P, 512], F32, tag="otile")
                        nc.vector.tensor_tensor(
                            ot[:], hold["t"][:], bo_sb[:, ec * 512:(ec + 1) * 512],
                            mybir.AluOpType.add,
                        )
                        nc.sync.dma_start(
                            out_d[q0:q0 + P, ec * 512:(ec + 1) * 512], ot[:]
                        )
                    return [run_a, run_b]

                # ---------------- K.T projection ------------------------
                for do in range(DO):
                    for ms in range(2):
                        kps = psB.tile([P, 512], F32, tag="big")
                        for ko in range(KO):
                            nc.tensor.matmul(
                                kps[:],
                                wk_sb[:, ko, do * P:(do + 1) * P],
                                ctxT[:, ko, ms * 512:(ms + 1) * 512],
                                start=(ko == 0), stop=(ko == KO - 1),
                            )
                        nc.vector.tensor_copy(kT[:, do, ms * 512:(ms + 1) * 512], kps[:])
                # Q.T for block 0 (so exps can start before V is ready)
                for do in range(DO):
                    for piece in qproj_pieces(0, do):
                        piece()
                # ---------------- V projection --------------------------
                for mo in range(MT):
                    vps = psB.tile([P, 512], F32, tag="big")
                    for ko in range(KO):
                        nc.tensor.matmul(
                            vps[:],
                            ctxT[:, ko, mo * P:(mo + 1) * P],
                            wv_sb[:, ko, :],
                            start=(ko == 0), stop=(ko == KO - 1),
                        )
                    nc.vector.tensor_copy(
                        v_sb[:, mo, :, 0:DH],
                        vps[:].rearrange("p (h d) -> p h d", h=HL),
                    )

                # ------ attention: per q-tile, finalize work of the ------
                # ------ previous q-tile interleaved into the head loop ---
                pending = []
                for qt in range(NT):
                    q0 = qt * P
                    o_sb = osbp.tile([P, HL, DH], F32, tag="osb")
                    for h in range(HL):
                        pb = (h % 2) * DH
                        sps = psS.tile([P, MT, P], F32, tag="s")
                        for mo in range(MT):
                            nc.tensor.matmul(
                                sps[:, mo],
                                kT[pb:pb + DH, h // 2, mo * P:(mo + 1) * P],
                                qT[pb:pb + DH, h // 2, q0:q0 + P],
                                start=True, stop=True,
                                skip_group_check=True,
                            )
                        if pending and h > 2:
                            pending.pop(0)()
                        if len(pending) >= 8:
                            pending.pop(0)()
                        ptile = ptp.tile([P, MT, P], BF16, tag="pt")
                        nc.scalar.activation(
                            ptile[:], sps[:],
                            mybir.ActivationFunctionType.Exp, scale=SCALE,
                        )
                        pv = psPV.tile([P, DH + 1], F32, tag="pv")
                        for mo in range(MT):
                            nc.tensor.matmul(
                                pv[:],
                                ptile[:, mo],
                                v_sb[:, mo, h, :],
                                start=(mo == 0), stop=(mo == MT - 1),
                                skip_group_check=True,
                            )
                        rec = recp.tile([P, 1], F32, tag="rec")
                        nc.vector.reciprocal(rec[:], pv[:, DH:DH + 1])
                        nc.vector.tensor_scalar_mul(o_sb[:, h, :], pv[:, 0:DH], rec[:])
                    pending.extend(transpose_pieces(q0, o_sb))
                    pending.extend(proj_pieces(q0, 0))
                    pending.extend(proj_pieces(q0, 1))
                    if qt % 4 == 1 and qt < 12:
                        # Q.T for the next 512-query block, ahead of its use
                        for do in range(DO):
                            pending.extend(qproj_pieces(qt // 4 + 1, do))
                for t in pending:
                    t()
    nc.finalize()
    return nc


def _get_nc():
    if "nc" not in _CACHE:
        _CACHE["nc"] = _build()
    return _CACHE["nc"]


def kernel(x, context, Wq, Wk, Wv, Wo, bo, **extra):
    nc = _get_nc()
    B = x.shape[0]
    bf = ml_dtypes.bfloat16
    ident = np.eye(P, dtype=np.float32)
    bo_b = np.broadcast_to(np.asarray(bo, dtype=np.float32), (P, E)).copy()
    zeros_bo = np.zeros((P, E), dtype=np.float32)
    x_b = np.asarray(x, dtype=bf)
    ctx_b = np.asarray(context, dtype=bf)
    wq_b = np.asarray(Wq, dtype=bf)
    wk_b = np.asarray(Wk, dtype=bf)
    wv_b = np.asarray(Wv, dtype=bf)
    wo_b = np.asarray(Wo, dtype=bf)
    in_maps = []
    for c in range(8):
        b, g = c // 2, c % 2
        in_maps.append({
            "x": np.ascontiguousarray(x_b[b]),
            "ctx": np.ascontiguousarray(ctx_b[b]),
            "wq": np.ascontiguousarray(wq_b[:, g * DHG:(g + 1) * DHG]),
            "wk": np.ascontiguousarray(wk_b[:, g * DHG:(g + 1) * DHG]),
            "wv": np.ascontiguousarray(wv_b[:, g * DHG:(g + 1) * DHG]),
            "wo": np.ascontiguousarray(wo_b[g * DHG:(g + 1) * DHG, :]),
            "bo": (bo_b if g == 0 else zeros_bo),
            "ident": ident,
        })
    global _last_in_maps
    _last_in_maps = in_maps
    res = run_bass_kernel_spmd(nc, in_maps, list(range(8)))
    out = np.empty((B, N, E), dtype=np.float32)
    for b in range(B):
        out[b] = res.results[2 * b]["out"] + res.results[2 * b + 1]["out"]
    return out


# revision 30
# speedup vs baseline: 1.0518x; 1.0101x over previous
"""Cross-attention Trainium2 kernel (8 NeuronCores, SPMD).

Sharding: core c handles batch c//2 and head-group c%2 (8 of 16 heads).
Each core computes its head-group's partial output projection; the host
sums the two partials per batch (bias is folded into head-group 0).

Shapes (hardcoded): B=4, N=2048 (queries), M=1024 (context), K=1024
(query/context dim), H=16 heads, DH=64, head-group width DHG=512, E=1024.

Dataflow (bf16 compute, fp32 PSUM accumulation / fp32 output):
  host pre-casts x/ctx/weights to bf16 and pre-broadcasts bias.
  xT/ctxT loaded via XBAR DMA transpose (no PE transposes for inputs).
  K.T = Wk.T @ ctxT, V = ctxT.T @ Wv, Q.T = Wq.T @ xT (bf16 matmuls).
  Per (q-tile of 128, head): S.T[m,q] (8 matmuls, d=64), P.T = exp on ACT
  (one 1024-wide activation per head), PV in [q-part, 65] orientation
  (V plus ones column -> softmax row sums land in column 64), reciprocal +
  per-partition-scalar normalize on DVE, O per q-tile PE-transposed back
  to O.T, out = O.T.T @ Wo; bias added by DVE during the PSUM->SBUF copy.
  The finalize work of each q-tile (transpose, output projection, store)
  and the next block's Q.T projection are split into ~450ns pieces and
  interleaved one-per-head into the following q-tile's S/exp/PV loop so
  PE stays busy during the exp latency of the S->exp->PV chain.
"""
import sys

if "/opt/trn_rl_repo" not in sys.path:
    sys.path.insert(0, "/opt/trn_rl_repo")

import numpy as np
import ml_dtypes

import concourse.bass as bass  # noqa: F401
import concourse.tile as tile
from concourse import bacc, mybir
from concourse.bass_utils import run_bass_kernel_spmd

P = 128
N = 2048          # queries per batch
M = 1024          # context rows
K = 1024          # query_dim == context_dim
DHG = 512         # d_attn per head group (8 heads x 64)
DH = 64           # dim per head
HL = 8            # heads per core
E = 1024          # output dim
SCALE = DH ** -0.5
F32 = mybir.dt.float32
BF16 = mybir.dt.bfloat16

KO = K // P       # 8 contraction chunks
NT = N // P       # 16 query tiles
MT = M // P       # 8 context tiles
DO = DHG // P     # 4 head-dim chunks
QC = N // 512     # 4 query blocks of 512
EC = E // 512     # 2 output chunks of 512

_CACHE = {}


def _build():
    nc = bacc.Bacc("TRN2", target_bir_lowering=False, debug=False, num_devices=8)
    x_d = nc.dram_tensor("x", [N, K], BF16, kind="ExternalInput")
    ctx_d = nc.dram_tensor("ctx", [M, K], BF16, kind="ExternalInput")
    wq_d = nc.dram_tensor("wq", [K, DHG], BF16, kind="ExternalInput")
    wk_d = nc.dram_tensor("wk", [K, DHG], BF16, kind="ExternalInput")
    wv_d = nc.dram_tensor("wv", [K, DHG], BF16, kind="ExternalInput")
    wo_d = nc.dram_tensor("wo", [DHG, E], BF16, kind="ExternalInput")
    bo_d = nc.dram_tensor("bo", [P, E], F32, kind="ExternalInput")
    id_d = nc.dram_tensor("ident", [P, P], F32, kind="ExternalInput")
    out_d = nc.dram_tensor("out", [N, E], F32, kind="ExternalOutput")

    with tile.TileContext(nc) as tc:
        with tc.tile_pool(name="persist", bufs=1) as pp:
            # DMA issue order tuned so K-proj (wk+ctxT) then Q-proj (wq+xT0)
            # inputs arrive first on the serialized DMA device.
            ident = pp.tile([P, P], F32)
            bo_sb = pp.tile([P, E], F32)
            wq_sb = pp.tile([P, KO, DHG], BF16)
            wk_sb = pp.tile([P, KO, DHG], BF16)
            wv_sb = pp.tile([P, KO, DHG], BF16)
            wo_sb = pp.tile([P, DO, E], BF16)
            ctxT = pp.tile([P, KO, M], BF16)
            xT = pp.tile([P, KO, N], BF16)

            nc.sync.dma_start(wk_sb[:], wk_d.rearrange("(ko p) d -> p ko d", p=P))
            for ms in range(2):
                nc.sync.dma_start_transpose(
                    ctxT[:, :, ms * 512:(ms + 1) * 512],
                    ctx_d[ms * 512:(ms + 1) * 512, :],
                )
            nc.sync.dma_start(wq_sb[:], wq_d.rearrange("(ko p) d -> p ko d", p=P))
            nc.sync.dma_start_transpose(xT[:, :, 0:512], x_d[0:512, :])
            nc.sync.dma_start(wv_sb[:], wv_d.rearrange("(ko p) d -> p ko d", p=P))
            nc.sync.dma_start(wo_sb[:], wo_d.rearrange("(do p) e -> p do e", p=P))
            for ns in range(1, QC):
                nc.sync.dma_start_transpose(
                    xT[:, :, ns * 512:(ns + 1) * 512],
                    x_d[ns * 512:(ns + 1) * 512, :],
                )
            nc.sync.dma_start(ident[:], id_d[:])
            nc.sync.dma_start(bo_sb[:], bo_d[:])

            kT = pp.tile([P, DO, M], BF16)        # K.T  [dhg, m]
            qT = pp.tile([P, DO, N], BF16)        # Q.T  [dhg, n]
            v_sb = pp.tile([P, MT, HL, DH + 1], BF16)  # V + ones col per head
            oT_sb = pp.tile([P, DO, N], BF16)     # O.T  [dhg, n] normalized
            nc.vector.memset(v_sb[:, :, :, DH], 1.0)

            with tc.tile_pool(name="psBig", bufs=2, space="PSUM") as psB, \
                 tc.tile_pool(name="psS", bufs=2, space="PSUM") as psS, \
                 tc.tile_pool(name="psPV", bufs=2, space="PSUM") as psPV, \
                 tc.tile_pool(name="pt", bufs=4) as ptp, \
                 tc.tile_pool(name="osb", bufs=2) as osbp, \
                 tc.tile_pool(name="rec", bufs=8) as recp, \
                 tc.tile_pool(name="od", bufs=4) as odp:
                def qproj_pieces(qb, do):
                    # two ~450ns PE pieces sharing one accumulation tile
                    hold = {}

                    def run_a():
                        hold["t"] = psB.tile([P, 512], F32, tag="big", name="qps")
                        for ko in range(4):
                            nc.tensor.matmul(
                                hold["t"][:],
                                wq_sb[:, ko, do * P:(do + 1) * P],
                                xT[:, ko, qb * 512:(qb + 1) * 512],
                                start=(ko == 0), stop=False,
                            )

                    def run_b():
                        for ko in range(4, KO):
                            nc.tensor.matmul(
                                hold["t"][:],
                                wq_sb[:, ko, do * P:(do + 1) * P],
                                xT[:, ko, qb * 512:(qb + 1) * 512],
                                start=False, stop=(ko == KO - 1),
                            )
                        nc.vector.tensor_copy(
                            qT[:, do, qb * 512:(qb + 1) * 512], hold["t"][:]
                        )
                    return [run_a, run_b]

                def transpose_pieces(q0, o_sb):
                    def run():
                        o_flat = o_sb[:].rearrange("p a b -> p (a b)")
                        otp = psB.tile([P, 512], F32, tag="big")
                        for d in range(DO):
                            nc.tensor.transpose(
                                otp[:, d * P:(d + 1) * P],
                                o_flat[:, d * P:(d + 1) * P],
                                ident[:],
                            )
                        nc.vector.tensor_copy(
                            oT_sb[:, :, q0:q0 + P],
                            otp[:].rearrange("p (a b) -> p a b", a=DO),
                        )
                    return [run]

                def proj_pieces(q0, ec):
                    hold = {}

                    def run_a():
                        hold["t"] = psB.tile([P, 512], F32, tag="big", name="fps")
                        for do in range(2):
                            nc.tensor.matmul(
                                hold["t"][:],
                                oT_sb[:, do, q0:q0 + P],
                                wo_sb[:, do, ec * 512:(ec + 1) * 512],
                                start=(do == 0), stop=False,
                            )

                    def run_b():
                        for do in range(2, DO):
                            nc.tensor.matmul(
                                hold["t"][:],
                                oT_sb[:, do, q0:q0 + P],
                                wo_sb[:, do, ec * 512:(ec + 1) * 512],
                                start=False, stop=(do == DO - 1),
                            )
                        ot = odp.tile([P, 512], F32, tag="otile")
                        nc.vector.tensor_tensor(
                            ot[:], hold["t"][:], bo_sb[:, ec * 512:(ec + 1) * 512],
                            mybir.AluOpType.add,
                        )
                        nc.sync.dma_start(
                            out_d[q0:q0 + P, ec * 512:(ec + 1) * 512], ot[:]
                        )
                    return [run_a, run_b]

                # ---------------- K.T projection ------------------------
                for do in range(DO):
                    for ms in range(2):
                        kps = psB.tile([P, 512], F32, tag="big")
                        for ko in range(KO):
                            nc.tensor.matmul(
                                kps[:],
                                wk_sb[:, ko, do * P:(do + 1) * P],
                                ctxT[:, ko, ms * 512:(ms + 1) * 512],
                                start=(ko == 0), stop=(ko == KO - 1),
                            )
                        nc.vector.tensor_copy(kT[:, do, ms * 512:(ms + 1) * 512], kps[:])
                # Q.T for block 0 (so exps can start before V is ready)
                for do in range(DO):
                    for piece in qproj_pieces(0, do):
                        piece()
                # ---------------- V projection --------------------------
                for mo in range(MT):
                    vps = psB.tile([P, 512], F32, tag="big")
                    for ko in range(KO):
                        nc.tensor.matmul(
                            vps[:],
                            ctxT[:, ko, mo * P:(mo + 1) * P],
                            wv_sb[:, ko, :],
                            start=(ko == 0), stop=(ko == KO - 1),
                        )
                    nc.vector.tensor_copy(
                        v_sb[:, mo, :, 0:DH],
                        vps[:].rearrange("p (h d) -> p h d", h=HL),
                    )

                # ------ attention: per q-tile, finalize work of the ------
                # ------ previous q-tile interleaved into the head loop ---
                pending = []
                for qt in range(NT):
                    q0 = qt * P
                    o_sb = osbp.tile([P, HL, DH], F32, tag="osb")
                    for h in range(HL):
                        pb = (h % 2) * DH
                        sps = psS.tile([P, MT, P], F32, tag="s")
                        for mo in range(MT):
                            nc.tensor.matmul(
                                sps[:, mo],
                                kT[pb:pb + DH, h // 2, mo * P:(mo + 1) * P],
                                qT[pb:pb + DH, h // 2, q0:q0 + P],
                                start=True, stop=True,
                                skip_group_check=True,
                            )
                        if pending and h > 3:
                            pending.pop(0)()
                        if len(pending) >= 8:
                            pending.pop(0)()
                        ptile = ptp.tile([P, MT, P], BF16, tag="pt")
                        nc.scalar.activation(
                            ptile[:], sps[:],
                            mybir.ActivationFunctionType.Exp, scale=SCALE,
                        )
                        pv = psPV.tile([P, DH + 1], F32, tag="pv")
                        for mo in range(MT):
                            nc.tensor.matmul(
                                pv[:],
                                ptile[:, mo],
                                v_sb[:, mo, h, :],
                                start=(mo == 0), stop=(mo == MT - 1),
                                skip_group_check=True,
                            )
                        rec = recp.tile([P, 1], F32, tag="rec")
                        nc.vector.reciprocal(rec[:], pv[:, DH:DH + 1])
                        nc.vector.tensor_scalar_mul(o_sb[:, h, :], pv[:, 0:DH], rec[:])
                    pending.extend(transpose_pieces(q0, o_sb))
                    pending.extend(proj_pieces(q0, 0))
                    pending.extend(proj_pieces(q0, 1))
                    if qt % 4 == 1 and qt < 12:
                        # Q.T for the next 512-query block, ahead of its use
                        for do in range(DO):
                            pending.extend(qproj_pieces(qt // 4 + 1, do))
                for t in pending:
                    t()
    nc.finalize()
    return nc


def _get_nc():
    if "nc" not in _CACHE:
        _CACHE["nc"] = _build()
    return _CACHE["nc"]


def kernel(x, context, Wq, Wk, Wv, Wo, bo, **extra):
    nc = _get_nc()
    B = x.shape[0]
    bf = ml_dtypes.bfloat16
    ident = np.eye(P, dtype=np.float32)
    bo_b = np.broadcast_to(np.asarray(bo, dtype=np.float32), (P, E)).copy()
    zeros_bo = np.zeros((P, E), dtype=np.float32)
    x_b = np.asarray(x, dtype=bf)
    ctx_b = np.asarray(context, dtype=bf)
    wq_b = np.asarray(Wq, dtype=bf)
    wk_b = np.asarray(Wk, dtype=bf)
    wv_b = np.asarray(Wv, dtype=bf)
    wo_b = np.asarray(Wo, dtype=bf)
    in_maps = []
    for c in range(8):
        b, g = c // 2, c % 2
        in_maps.append({
            "x": np.ascontiguousarray(x_b[b]),
            "ctx": np.ascontiguousarray(ctx_b[b]),
            "wq": np.ascontiguousarray(wq_b[:, g * DHG:(g + 1) * DHG]),
            "wk": np.ascontiguousarray(wk_b[:, g * DHG:(g + 1) * DHG]),
            "wv": np.ascontiguousarray(wv_b[:, g * DHG:(g + 1) * DHG]),
            "wo": np.ascontiguousarray(wo_b[g * DHG:(g + 1) * DHG, :]),
            "bo": (bo_b if g == 0 else zeros_bo),
            "ident": ident,
        })
    global _last_in_maps
    _last_in_maps = in_maps
    res = run_bass_kernel_spmd(nc, in_maps, list(range(8)))
    out = np.empty((B, N, E), dtype=np.float32)
    for b in range(B):
        out[b] = res.results[2 * b]["out"] + res.results[2 * b + 1]["out"]
    return out


# revision 31
# speedup vs baseline: 1.0538x; 1.0018x over previous
"""Cross-attention Trainium2 kernel (8 NeuronCores, SPMD).

Sharding: core c handles batch c//2 and head-group c%2 (8 of 16 heads).
Each core computes its head-group's partial output projection; the host
sums the two partials per batch (bias is folded into head-group 0).

Shapes (hardcoded): B=4, N=2048 (queries), M=1024 (context), K=1024
(query/context dim), H=16 heads, DH=64, head-group width DHG=512, E=1024.

Dataflow (bf16 compute, fp32 PSUM accumulation / fp32 output):
  host pre-casts x/ctx/weights to bf16 and pre-broadcasts bias.
  xT/ctxT loaded via XBAR DMA transpose (no PE transposes for inputs).
  K.T = Wk.T @ ctxT, V = ctxT.T @ Wv, Q.T = Wq.T @ xT (bf16 matmuls).
  Per (q-tile of 128, head): S.T[m,q] (8 matmuls, d=64), P.T = exp on ACT
  (one 1024-wide activation per head), PV in [q-part, 65] orientation
  (V plus ones column -> softmax row sums land in column 64), reciprocal +
  per-partition-scalar normalize on DVE, O per q-tile PE-transposed back
  to O.T, out = O.T.T @ Wo; bias added by DVE during the PSUM->SBUF copy.
  The finalize work of each q-tile (transpose, output projection, store)
  and the next block's Q.T projection are split into ~450ns pieces and
  interleaved one-per-head into the following q-tile's S/exp/PV loop so
  PE stays busy during the exp latency of the S->exp->PV chain.
"""
import sys

if "/opt/trn_rl_repo" not in sys.path:
    sys.path.insert(0, "/opt/trn_rl_repo")

import numpy as np
import ml_dtypes

import concourse.bass as bass  # noqa: F401
import concourse.tile as tile
from concourse import bacc, mybir
from concourse.bass_utils import run_bass_kernel_spmd

P = 128
N = 2048          # queries per batch
M = 1024          # context rows
K = 1024          # query_dim == context_dim
DHG = 512         # d_attn per head group (8 heads x 64)
DH = 64           # dim per head
HL = 8            # heads per core
E = 1024          # output dim
SCALE = DH ** -0.5
F32 = mybir.dt.float32
BF16 = mybir.dt.bfloat16

KO = K // P       # 8 contraction chunks
NT = N // P       # 16 query tiles
MT = M // P       # 8 context tiles
DO = DHG // P     # 4 head-dim chunks
QC = N // 512     # 4 query blocks of 512
EC = E // 512     # 2 output chunks of 512

_CACHE = {}


def _build():
    nc = bacc.Bacc("TRN2", target_bir_lowering=False, debug=False, num_devices=8)
    x_d = nc.dram_tensor("x", [N, K], BF16, kind="ExternalInput")
    ctx_d = nc.dram_tensor("ctx", [M, K], BF16, kind="ExternalInput")
    wq_d = nc.dram_tensor("wq", [K, DHG], BF16, kind="ExternalInput")
    wk_d = nc.dram_tensor("wk", [K, DHG], BF16, kind="ExternalInput")
    wv_d = nc.dram_tensor("wv", [K, DHG], BF16, kind="ExternalInput")
    wo_d = nc.dram_tensor("wo", [DHG, E], BF16, kind="ExternalInput")
    bo_d = nc.dram_tensor("bo", [P, E], F32, kind="ExternalInput")
    id_d = nc.dram_tensor("ident", [P, P], F32, kind="ExternalInput")
    out_d = nc.dram_tensor("out", [N, E], F32, kind="ExternalOutput")

    with tile.TileContext(nc) as tc:
        with tc.tile_pool(name="persist", bufs=1) as pp:
            # DMA issue order tuned so K-proj (wk+ctxT) then Q-proj (wq+xT0)
            # inputs arrive first on the serialized DMA device.
            ident = pp.tile([P, P], F32)
            bo_sb = pp.tile([P, E], F32)
            wq_sb = pp.tile([P, KO, DHG], BF16)
            wk_sb = pp.tile([P, KO, DHG], BF16)
            wv_sb = pp.tile([P, KO, DHG], BF16)
            wo_sb = pp.tile([P, DO, E], BF16)
            ctxT = pp.tile([P, KO, M], BF16)
            xT = pp.tile([P, KO, N], BF16)

            nc.sync.dma_start(wk_sb[:], wk_d.rearrange("(ko p) d -> p ko d", p=P))
            for ms in range(2):
                nc.sync.dma_start_transpose(
                    ctxT[:, :, ms * 512:(ms + 1) * 512],
                    ctx_d[ms * 512:(ms + 1) * 512, :],
                )
            nc.sync.dma_start(wq_sb[:], wq_d.rearrange("(ko p) d -> p ko d", p=P))
            nc.sync.dma_start_transpose(xT[:, :, 0:512], x_d[0:512, :])
            nc.sync.dma_start(wv_sb[:], wv_d.rearrange("(ko p) d -> p ko d", p=P))
            nc.sync.dma_start(wo_sb[:], wo_d.rearrange("(do p) e -> p do e", p=P))
            for ns in range(1, QC):
                nc.sync.dma_start_transpose(
                    xT[:, :, ns * 512:(ns + 1) * 512],
                    x_d[ns * 512:(ns + 1) * 512, :],
                )
            nc.sync.dma_start(ident[:], id_d[:])
            nc.sync.dma_start(bo_sb[:], bo_d[:])

            kT = pp.tile([P, DO, M], BF16)        # K.T  [dhg, m]
            qT = pp.tile([P, DO, N], BF16)        # Q.T  [dhg, n]
            v_sb = pp.tile([P, MT, HL, DH + 1], BF16)  # V + ones col per head
            oT_sb = pp.tile([P, DO, N], BF16)     # O.T  [dhg, n] normalized
            nc.vector.memset(v_sb[:, :, :, DH], 1.0)

            with tc.tile_pool(name="psBig", bufs=2, space="PSUM") as psB, \
                 tc.tile_pool(name="psS", bufs=2, space="PSUM") as psS, \
                 tc.tile_pool(name="psPV", bufs=2, space="PSUM") as psPV, \
                 tc.tile_pool(name="pt", bufs=4) as ptp, \
                 tc.tile_pool(name="osb", bufs=2) as osbp, \
                 tc.tile_pool(name="rec", bufs=8) as recp, \
                 tc.tile_pool(name="od", bufs=4) as odp:
                def qproj_pieces(qb, do):
                    # two ~450ns PE pieces sharing one accumulation tile
                    hold = {}

                    def run_a():
                        hold["t"] = psB.tile([P, 512], F32, tag="big", name="qps")
                        for ko in range(4):
                            nc.tensor.matmul(
                                hold["t"][:],
                                wq_sb[:, ko, do * P:(do + 1) * P],
                                xT[:, ko, qb * 512:(qb + 1) * 512],
                                start=(ko == 0), stop=False,
                            )

                    def run_b():
                        for ko in range(4, KO):
                            nc.tensor.matmul(
                                hold["t"][:],
                                wq_sb[:, ko, do * P:(do + 1) * P],
                                xT[:, ko, qb * 512:(qb + 1) * 512],
                                start=False, stop=(ko == KO - 1),
                            )
                        nc.vector.tensor_copy(
                            qT[:, do, qb * 512:(qb + 1) * 512], hold["t"][:]
                        )
                    return [run_a, run_b]

                def transpose_pieces(q0, o_sb):
                    def run():
                        o_flat = o_sb[:].rearrange("p a b -> p (a b)")
                        otp = psB.tile([P, 512], F32, tag="big")
                        for d in range(DO):
                            nc.tensor.transpose(
                                otp[:, d * P:(d + 1) * P],
                                o_flat[:, d * P:(d + 1) * P],
                                ident[:],
                            )
                        nc.vector.tensor_copy(
                            oT_sb[:, :, q0:q0 + P],
                            otp[:].rearrange("p (a b) -> p a b", a=DO),
                        )
                    return [run]

                def proj_pieces(q0, ec):
                    hold = {}

                    def run_a():
                        hold["t"] = psB.tile([P, 512], F32, tag="big", name="fps")
                        for do in range(2):
                            nc.tensor.matmul(
                                hold["t"][:],
                                oT_sb[:, do, q0:q0 + P],
                                wo_sb[:, do, ec * 512:(ec + 1) * 512],
                                start=(do == 0), stop=False,
                            )

                    def run_b():
                        for do in range(2, DO):
                            nc.tensor.matmul(
                                hold["t"][:],
                                oT_sb[:, do, q0:q0 + P],
                                wo_sb[:, do, ec * 512:(ec + 1) * 512],
                                start=False, stop=(do == DO - 1),
                            )
                        ot = odp.tile([P, 512], F32, tag="otile")
                        nc.vector.tensor_tensor(
                            ot[:], hold["t"][:], bo_sb[:, ec * 512:(ec + 1) * 512],
                            mybir.AluOpType.add,
                        )
                        nc.sync.dma_start(
                            out_d[q0:q0 + P, ec * 512:(ec + 1) * 512], ot[:]
                        )
                    return [run_a, run_b]

                # ---------------- K.T projection ------------------------
                for do in range(DO):
                    for ms in range(2):
                        kps = psB.tile([P, 512], F32, tag="big")
                        for ko in range(KO):
                            nc.tensor.matmul(
                                kps[:],
                                wk_sb[:, ko, do * P:(do + 1) * P],
                                ctxT[:, ko, ms * 512:(ms + 1) * 512],
                                start=(ko == 0), stop=(ko == KO - 1),
                            )
                        nc.vector.tensor_copy(kT[:, do, ms * 512:(ms + 1) * 512], kps[:])
                # Q.T for block 0 (so exps can start before V is ready)
                for do in range(DO):
                    for piece in qproj_pieces(0, do):
                        piece()
                # ---------------- V projection --------------------------
                for mo in range(MT):
                    vps = psB.tile([P, 512], F32, tag="big")
                    for ko in range(KO):
                        nc.tensor.matmul(
                            vps[:],
                            ctxT[:, ko, mo * P:(mo + 1) * P],
                            wv_sb[:, ko, :],
                            start=(ko == 0), stop=(ko == KO - 1),
                        )
                    nc.vector.tensor_copy(
                        v_sb[:, mo, :, 0:DH],
                        vps[:].rearrange("p (h d) -> p h d", h=HL),
                    )

                # ------ attention: per q-tile, finalize work of the ------
                # ------ previous q-tile interleaved into the head loop ---
                pending = []
                for qt in range(NT):
                    q0 = qt * P
                    o_sb = osbp.tile([P, HL, DH], F32, tag="osb")
                    for h in range(HL):
                        pb = (h % 2) * DH
                        sps = psS.tile([P, MT, P], F32, tag="s")
                        for mo in range(MT):
                            nc.tensor.matmul(
                                sps[:, mo],
                                kT[pb:pb + DH, h // 2, mo * P:(mo + 1) * P],
                                qT[pb:pb + DH, h // 2, q0:q0 + P],
                                start=True, stop=True,
                                skip_group_check=True,
                            )
                        if pending and h > 4:
                            pending.pop(0)()
                        if len(pending) >= 8:
                            pending.pop(0)()
                        ptile = ptp.tile([P, MT, P], BF16, tag="pt")
                        nc.scalar.activation(
                            ptile[:], sps[:],
                            mybir.ActivationFunctionType.Exp, scale=SCALE,
                        )
                        pv = psPV.tile([P, DH + 1], F32, tag="pv")
                        for mo in range(MT):
                            nc.tensor.matmul(
                                pv[:],
                                ptile[:, mo],
                                v_sb[:, mo, h, :],
                                start=(mo == 0), stop=(mo == MT - 1),
                                skip_group_check=True,
                            )
                        rec = recp.tile([P, 1], F32, tag="rec")
                        nc.vector.reciprocal(rec[:], pv[:, DH:DH + 1])
                        nc.vector.tensor_scalar_mul(o_sb[:, h, :], pv[:, 0:DH], rec[:])
                    pending.extend(transpose_pieces(q0, o_sb))
                    pending.extend(proj_pieces(q0, 0))
                    pending.extend(proj_pieces(q0, 1))
                    if qt % 4 == 1 and qt < 12:
                        # Q.T for the next 512-query block, ahead of its use
                        for do in range(DO):
                            pending.extend(qproj_pieces(qt // 4 + 1, do))
                for t in pending:
                    t()
    nc.finalize()
    return nc


def _get_nc():
    if "nc" not in _CACHE:
        _CACHE["nc"] = _build()
    return _CACHE["nc"]


def kernel(x, context, Wq, Wk, Wv, Wo, bo, **extra):
    nc = _get_nc()
    B = x.shape[0]
    bf = ml_dtypes.bfloat16
    ident = np.eye(P, dtype=np.float32)
    bo_b = np.broadcast_to(np.asarray(bo, dtype=np.float32), (P, E)).copy()
    zeros_bo = np.zeros((P, E), dtype=np.float32)
    x_b = np.asarray(x, dtype=bf)
    ctx_b = np.asarray(context, dtype=bf)
    wq_b = np.asarray(Wq, dtype=bf)
    wk_b = np.asarray(Wk, dtype=bf)
    wv_b = np.asarray(Wv, dtype=bf)
    wo_b = np.asarray(Wo, dtype=bf)
    in_maps = []
    for c in range(8):
        b, g = c // 2, c % 2
        in_maps.append({
            "x": np.ascontiguousarray(x_b[b]),
            "ctx": np.ascontiguousarray(ctx_b[b]),
            "wq": np.ascontiguousarray(wq_b[:, g * DHG:(g + 1) * DHG]),
            "wk": np.ascontiguousarray(wk_b[:, g * DHG:(g + 1) * DHG]),
            "wv": np.ascontiguousarray(wv_b[:, g * DHG:(g + 1) * DHG]),
            "wo": np.ascontiguousarray(wo_b[g * DHG:(g + 1) * DHG, :]),
            "bo": (bo_b if g == 0 else zeros_bo),
            "ident": ident,
        })
    global _last_in_maps
    _last_in_maps = in_maps
    res = run_bass_kernel_spmd(nc, in_maps, list(range(8)))
    out = np.empty((B, N, E), dtype=np.float32)
    for b in range(B):
        out[b] = res.results[2 * b]["out"] + res.results[2 * b + 1]["out"]
    return out


# revision 32
# speedup vs baseline: 1.0586x; 1.0046x over previous
"""Cross-attention Trainium2 kernel (8 NeuronCores, SPMD).

Sharding: core c handles batch c//2 and head-group c%2 (8 of 16 heads).
Each core computes its head-group's partial output projection; the host
sums the two partials per batch (bias is folded into head-group 0).

Shapes (hardcoded): B=4, N=2048 (queries), M=1024 (context), K=1024
(query/context dim), H=16 heads, DH=64, head-group width DHG=512, E=1024.

Dataflow (bf16 compute, fp32 PSUM accumulation / fp32 output):
  host pre-casts x/ctx/weights to bf16 and pre-broadcasts bias.
  xT/ctxT loaded via XBAR DMA transpose (no PE transposes for inputs).
  K.T = Wk.T @ ctxT, V = ctxT.T @ Wv, Q.T = Wq.T @ xT (bf16 matmuls).
  Per (q-tile of 128, head): S.T[m,q] (8 matmuls, d=64), P.T = exp on ACT
  (one 1024-wide activation per head), PV in [q-part, 65] orientation
  (V plus ones column -> softmax row sums land in column 64), reciprocal +
  per-partition-scalar normalize on DVE, O per q-tile PE-transposed back
  to O.T, out = O.T.T @ Wo; bias added by DVE during the PSUM->SBUF copy.
  The finalize work of each q-tile (transpose, output projection, store)
  and the next block's Q.T projection are split into ~450ns pieces and
  interleaved one-per-head into the following q-tile's S/exp/PV loop so
  PE stays busy during the exp latency of the S->exp->PV chain.
"""
import sys

if "/opt/trn_rl_repo" not in sys.path:
    sys.path.insert(0, "/opt/trn_rl_repo")

import numpy as np
import ml_dtypes

import concourse.bass as bass  # noqa: F401
import concourse.tile as tile
from concourse import bacc, mybir
from concourse.bass_utils import run_bass_kernel_spmd

P = 128
N = 2048          # queries per batch
M = 1024          # context rows
K = 1024          # query_dim == context_dim
DHG = 512         # d_attn per head group (8 heads x 64)
DH = 64           # dim per head
HL = 8            # heads per core
E = 1024          # output dim
SCALE = DH ** -0.5
F32 = mybir.dt.float32
BF16 = mybir.dt.bfloat16

KO = K // P       # 8 contraction chunks
NT = N // P       # 16 query tiles
MT = M // P       # 8 context tiles
DO = DHG // P     # 4 head-dim chunks
QC = N // 512     # 4 query blocks of 512
EC = E // 512     # 2 output chunks of 512

_CACHE = {}


def _build():
    nc = bacc.Bacc("TRN2", target_bir_lowering=False, debug=False, num_devices=8)
    x_d = nc.dram_tensor("x", [N, K], BF16, kind="ExternalInput")
    ctx_d = nc.dram_tensor("ctx", [M, K], BF16, kind="ExternalInput")
    wq_d = nc.dram_tensor("wq", [K, DHG], BF16, kind="ExternalInput")
    wk_d = nc.dram_tensor("wk", [K, DHG], BF16, kind="ExternalInput")
    wv_d = nc.dram_tensor("wv", [K, DHG], BF16, kind="ExternalInput")
    wo_d = nc.dram_tensor("wo", [DHG, E], BF16, kind="ExternalInput")
    bo_d = nc.dram_tensor("bo", [P, E], F32, kind="ExternalInput")
    id_d = nc.dram_tensor("ident", [P, P], F32, kind="ExternalInput")
    out_d = nc.dram_tensor("out", [N, E], F32, kind="ExternalOutput")

    with tile.TileContext(nc) as tc:
        with tc.tile_pool(name="persist", bufs=1) as pp:
            # DMA issue order tuned so K-proj (wk+ctxT) then Q-proj (wq+xT0)
            # inputs arrive first on the serialized DMA device.
            ident = pp.tile([P, P], F32)
            bo_sb = pp.tile([P, E], F32)
            wq_sb = pp.tile([P, KO, DHG], BF16)
            wk_sb = pp.tile([P, KO, DHG], BF16)
            wv_sb = pp.tile([P, KO, DHG], BF16)
            wo_sb = pp.tile([P, DO, E], BF16)
            ctxT = pp.tile([P, KO, M], BF16)
            xT = pp.tile([P, KO, N], BF16)

            nc.sync.dma_start(wk_sb[:], wk_d.rearrange("(ko p) d -> p ko d", p=P))
            for ms in range(2):
                nc.sync.dma_start_transpose(
                    ctxT[:, :, ms * 512:(ms + 1) * 512],
                    ctx_d[ms * 512:(ms + 1) * 512, :],
                )
            nc.sync.dma_start(wq_sb[:], wq_d.rearrange("(ko p) d -> p ko d", p=P))
            nc.sync.dma_start_transpose(xT[:, :, 0:512], x_d[0:512, :])
            nc.sync.dma_start(wv_sb[:], wv_d.rearrange("(ko p) d -> p ko d", p=P))
            nc.sync.dma_start(wo_sb[:], wo_d.rearrange("(do p) e -> p do e", p=P))
            for ns in range(1, QC):
                nc.sync.dma_start_transpose(
                    xT[:, :, ns * 512:(ns + 1) * 512],
                    x_d[ns * 512:(ns + 1) * 512, :],
                )
            nc.sync.dma_start(ident[:], id_d[:])
            nc.sync.dma_start(bo_sb[:], bo_d[:])

            kT = pp.tile([P, DO, M], BF16)        # K.T  [dhg, m]
            qT = pp.tile([P, DO, N], BF16)        # Q.T  [dhg, n]
            v_sb = pp.tile([P, MT, HL, DH + 1], BF16)  # V + ones col per head
            oT_sb = pp.tile([P, DO, N], BF16)     # O.T  [dhg, n] normalized
            nc.vector.memset(v_sb[:, :, :, DH], 1.0)

            with tc.tile_pool(name="psBig", bufs=2, space="PSUM") as psB, \
                 tc.tile_pool(name="psS", bufs=2, space="PSUM") as psS, \
                 tc.tile_pool(name="psPV", bufs=2, space="PSUM") as psPV, \
                 tc.tile_pool(name="pt", bufs=4) as ptp, \
                 tc.tile_pool(name="osb", bufs=2) as osbp, \
                 tc.tile_pool(name="rec", bufs=8) as recp, \
                 tc.tile_pool(name="od", bufs=4) as odp:
                def qproj_pieces(qb, do):
                    # two ~450ns PE pieces sharing one accumulation tile
                    hold = {}

                    def run_a():
                        hold["t"] = psB.tile([P, 512], F32, tag="big", name="qps")
                        for ko in range(4):
                            nc.tensor.matmul(
                                hold["t"][:],
                                wq_sb[:, ko, do * P:(do + 1) * P],
                                xT[:, ko, qb * 512:(qb + 1) * 512],
                                start=(ko == 0), stop=False,
                            )

                    def run_b():
                        for ko in range(4, KO):
                            nc.tensor.matmul(
                                hold["t"][:],
                                wq_sb[:, ko, do * P:(do + 1) * P],
                                xT[:, ko, qb * 512:(qb + 1) * 512],
                                start=False, stop=(ko == KO - 1),
                            )
                        nc.vector.tensor_copy(
                            qT[:, do, qb * 512:(qb + 1) * 512], hold["t"][:]
                        )
                    return [run_a, run_b]

                def transpose_pieces(q0, o_sb):
                    def run():
                        o_flat = o_sb[:].rearrange("p a b -> p (a b)")
                        otp = psB.tile([P, 512], F32, tag="big")
                        for d in range(DO):
                            nc.tensor.transpose(
                                otp[:, d * P:(d + 1) * P],
                                o_flat[:, d * P:(d + 1) * P],
                                ident[:],
                            )
                        nc.vector.tensor_copy(
                            oT_sb[:, :, q0:q0 + P],
                            otp[:].rearrange("p (a b) -> p a b", a=DO),
                        )
                    return [run]

                def proj_pieces(q0, ec):
                    hold = {}

                    def run_a():
                        hold["t"] = psB.tile([P, 512], F32, tag="big", name="fps")
                        for do in range(2):
                            nc.tensor.matmul(
                                hold["t"][:],
                                oT_sb[:, do, q0:q0 + P],
                                wo_sb[:, do, ec * 512:(ec + 1) * 512],
                                start=(do == 0), stop=False,
                            )

                    def run_b():
                        for do in range(2, DO):
                            nc.tensor.matmul(
                                hold["t"][:],
                                oT_sb[:, do, q0:q0 + P],
                                wo_sb[:, do, ec * 512:(ec + 1) * 512],
                                start=False, stop=(do == DO - 1),
                            )
                        ot = odp.tile([P, 512], F32, tag="otile")
                        nc.vector.tensor_tensor(
                            ot[:], hold["t"][:], bo_sb[:, ec * 512:(ec + 1) * 512],
                            mybir.AluOpType.add,
                        )
                        nc.sync.dma_start(
                            out_d[q0:q0 + P, ec * 512:(ec + 1) * 512], ot[:]
                        )
                    return [run_a, run_b]

                # ---------------- K.T projection ------------------------
                for do in range(DO):
                    for ms in range(2):
                        kps = psB.tile([P, 512], F32, tag="big")
                        for ko in range(KO):
                            nc.tensor.matmul(
                                kps[:],
                                wk_sb[:, ko, do * P:(do + 1) * P],
                                ctxT[:, ko, ms * 512:(ms + 1) * 512],
                                start=(ko == 0), stop=(ko == KO - 1),
                            )
                        nc.vector.tensor_copy(kT[:, do, ms * 512:(ms + 1) * 512], kps[:])
                # Q.T for block 0 (so exps can start before V is ready)
                for do in range(DO):
                    for piece in qproj_pieces(0, do):
                        piece()
                # ---------------- V projection --------------------------
                for mo in range(MT):
                    vps = psB.tile([P, 512], F32, tag="big")
                    for ko in range(KO):
                        nc.tensor.matmul(
                            vps[:],
                            ctxT[:, ko, mo * P:(mo + 1) * P],
                            wv_sb[:, ko, :],
                            start=(ko == 0), stop=(ko == KO - 1),
                        )
                    nc.vector.tensor_copy(
                        v_sb[:, mo, :, 0:DH],
                        vps[:].rearrange("p (h d) -> p h d", h=HL),
                    )

                # ------ attention: per q-tile, finalize work of the ------
                # ------ previous q-tile interleaved into the head loop ---
                pending = []
                for qt in range(NT):
                    q0 = qt * P
                    o_sb = osbp.tile([P, HL, DH], F32, tag="osb")
                    for h in range(HL):
                        pb = (h % 2) * DH
                        sps = psS.tile([P, MT, P], F32, tag="s")
                        for mo in range(MT):
                            nc.tensor.matmul(
                                sps[:, mo],
                                kT[pb:pb + DH, h // 2, mo * P:(mo + 1) * P],
                                qT[pb:pb + DH, h // 2, q0:q0 + P],
                                start=True, stop=True,
                                skip_group_check=True,
                            )
                        if pending and h > 5:
                            pending.pop(0)()
                        if len(pending) >= 8:
                            pending.pop(0)()
                        ptile = ptp.tile([P, MT, P], BF16, tag="pt")
                        nc.scalar.activation(
                            ptile[:], sps[:],
                            mybir.ActivationFunctionType.Exp, scale=SCALE,
                        )
                        pv = psPV.tile([P, DH + 1], F32, tag="pv")
                        for mo in range(MT):
                            nc.tensor.matmul(
                                pv[:],
                                ptile[:, mo],
                                v_sb[:, mo, h, :],
                                start=(mo == 0), stop=(mo == MT - 1),
                                skip_group_check=True,
                            )
                        rec = recp.tile([P, 1], F32, tag="rec")
                        nc.vector.reciprocal(rec[:], pv[:, DH:DH + 1])
                        nc.vector.tensor_scalar_mul(o_sb[:, h, :], pv[:, 0:DH], rec[:])
                    pending.extend(transpose_pieces(q0, o_sb))
                    pending.extend(proj_pieces(q0, 0))
                    pending.extend(proj_pieces(q0, 1))
                    if qt % 4 == 1 and qt < 12:
                        # Q.T for the next 512-query block, ahead of its use
                        for do in range(DO):
                            pending.extend(qproj_pieces(qt // 4 + 1, do))
                for t in pending:
                    t()
    nc.finalize()
    return nc


def _get_nc():
    if "nc" not in _CACHE:
        _CACHE["nc"] = _build()
    return _CACHE["nc"]


def kernel(x, context, Wq, Wk, Wv, Wo, bo, **extra):
    nc = _get_nc()
    B = x.shape[0]
    bf = ml_dtypes.bfloat16
    ident = np.eye(P, dtype=np.float32)
    bo_b = np.broadcast_to(np.asarray(bo, dtype=np.float32), (P, E)).copy()
    zeros_bo = np.zeros((P, E), dtype=np.float32)
    x_b = np.asarray(x, dtype=bf)
    ctx_b = np.asarray(context, dtype=bf)
    wq_b = np.asarray(Wq, dtype=bf)
    wk_b = np.asarray(Wk, dtype=bf)
    wv_b = np.asarray(Wv, dtype=bf)
    wo_b = np.asarray(Wo, dtype=bf)
    in_maps = []
    for c in range(8):
        b, g = c // 2, c % 2
        in_maps.append({
            "x": np.ascontiguousarray(x_b[b]),
            "ctx": np.ascontiguousarray(ctx_b[b]),
            "wq": np.ascontiguousarray(wq_b[:, g * DHG:(g + 1) * DHG]),
            "wk": np.ascontiguousarray(wk_b[:, g * DHG:(g + 1) * DHG]),
            "wv": np.ascontiguousarray(wv_b[:, g * DHG:(g + 1) * DHG]),
            "wo": np.ascontiguousarray(wo_b[g * DHG:(g + 1) * DHG, :]),
            "bo": (bo_b if g == 0 else zeros_bo),
            "ident": ident,
        })
    global _last_in_maps
    _last_in_maps = in_maps
    res = run_bass_kernel_spmd(nc, in_maps, list(range(8)))
    out = np.empty((B, N, E), dtype=np.float32)
    for b in range(B):
        out[b] = res.results[2 * b]["out"] + res.results[2 * b + 1]["out"]
    return out


# revision 33
# speedup vs baseline: 1.0622x; 1.0034x over previous
"""Cross-attention Trainium2 kernel (8 NeuronCores, SPMD).

Sharding: core c handles batch c//2 and head-group c%2 (8 of 16 heads).
Each core computes its head-group's partial output projection; the host
sums the two partials per batch (bias is folded into head-group 0).

Shapes (hardcoded): B=4, N=2048 (queries), M=1024 (context), K=1024
(query/context dim), H=16 heads, DH=64, head-group width DHG=512, E=1024.

Dataflow (bf16 compute, fp32 PSUM accumulation / fp32 output):
  host pre-casts x/ctx/weights to bf16 and pre-broadcasts bias.
  xT/ctxT loaded via XBAR DMA transpose (no PE transposes for inputs).
  K.T = Wk.T @ ctxT, V = ctxT.T @ Wv, Q.T = Wq.T @ xT (bf16 matmuls).
  Per (q-tile of 128, head): S.T[m,q] (8 matmuls, d=64), P.T = exp on ACT
  (one 1024-wide activation per head), PV in [q-part, 65] orientation
  (V plus ones column -> softmax row sums land in column 64), reciprocal +
  per-partition-scalar normalize on DVE, O per q-tile PE-transposed back
  to O.T, out = O.T.T @ Wo; bias added by DVE during the PSUM->SBUF copy.
  The finalize work of each q-tile (transpose, output projection, store)
  and the next block's Q.T projection are split into ~450ns pieces and
  interleaved one-per-head into the following q-tile's S/exp/PV loop so
  PE stays busy during the exp latency of the S->exp->PV chain.
"""
import sys

if "/opt/trn_rl_repo" not in sys.path:
    sys.path.insert(0, "/opt/trn_rl_repo")

import numpy as np
import ml_dtypes

import concourse.bass as bass  # noqa: F401
import concourse.tile as tile
from concourse import bacc, mybir
from concourse.bass_utils import run_bass_kernel_spmd

P = 128
N = 2048          # queries per batch
M = 1024          # context rows
K = 1024          # query_dim == context_dim
DHG = 512         # d_attn per head group (8 heads x 64)
DH = 64           # dim per head
HL = 8            # heads per core
E = 1024          # output dim
SCALE = DH ** -0.5
F32 = mybir.dt.float32
BF16 = mybir.dt.bfloat16

KO = K // P       # 8 contraction chunks
NT = N // P       # 16 query tiles
MT = M // P       # 8 context tiles
DO = DHG // P     # 4 head-dim chunks
QC = N // 512     # 4 query blocks of 512
EC = E // 512     # 2 output chunks of 512

_CACHE = {}


def _build():
    nc = bacc.Bacc("TRN2", target_bir_lowering=False, debug=False, num_devices=8)
    x_d = nc.dram_tensor("x", [N, K], BF16, kind="ExternalInput")
    ctx_d = nc.dram_tensor("ctx", [M, K], BF16, kind="ExternalInput")
    wq_d = nc.dram_tensor("wq", [K, DHG], BF16, kind="ExternalInput")
    wk_d = nc.dram_tensor("wk", [K, DHG], BF16, kind="ExternalInput")
    wv_d = nc.dram_tensor("wv", [K, DHG], BF16, kind="ExternalInput")
    wo_d = nc.dram_tensor("wo", [DHG, E], BF16, kind="ExternalInput")
    bo_d = nc.dram_tensor("bo", [P, E], F32, kind="ExternalInput")
    id_d = nc.dram_tensor("ident", [P, P], F32, kind="ExternalInput")
    out_d = nc.dram_tensor("out", [N, E], F32, kind="ExternalOutput")

    with tile.TileContext(nc) as tc:
        with tc.tile_pool(name="persist", bufs=1) as pp:
            # DMA issue order tuned so K-proj (wk+ctxT) then Q-proj (wq+xT0)
            # inputs arrive first on the serialized DMA device.
            ident = pp.tile([P, P], F32)
            bo_sb = pp.tile([P, E], F32)
            wq_sb = pp.tile([P, KO, DHG], BF16)
            wk_sb = pp.tile([P, KO, DHG], BF16)
            wv_sb = pp.tile([P, KO, DHG], BF16)
            wo_sb = pp.tile([P, DO, E], BF16)
            ctxT = pp.tile([P, KO, M], BF16)
            xT = pp.tile([P, KO, N], BF16)

            nc.sync.dma_start(wk_sb[:], wk_d.rearrange("(ko p) d -> p ko d", p=P))
            for ms in range(2):
                nc.sync.dma_start_transpose(
                    ctxT[:, :, ms * 512:(ms + 1) * 512],
                    ctx_d[ms * 512:(ms + 1) * 512, :],
                )
            nc.sync.dma_start(wq_sb[:], wq_d.rearrange("(ko p) d -> p ko d", p=P))
            nc.sync.dma_start_transpose(xT[:, :, 0:512], x_d[0:512, :])
            nc.sync.dma_start(wv_sb[:], wv_d.rearrange("(ko p) d -> p ko d", p=P))
            nc.sync.dma_start(wo_sb[:], wo_d.rearrange("(do p) e -> p do e", p=P))
            for ns in range(1, QC):
                nc.sync.dma_start_transpose(
                    xT[:, :, ns * 512:(ns + 1) * 512],
                    x_d[ns * 512:(ns + 1) * 512, :],
                )
            nc.sync.dma_start(ident[:], id_d[:])
            nc.sync.dma_start(bo_sb[:], bo_d[:])

            kT = pp.tile([P, DO, M], BF16)        # K.T  [dhg, m]
            qT = pp.tile([P, DO, N], BF16)        # Q.T  [dhg, n]
            v_sb = pp.tile([P, MT, HL, DH + 1], BF16)  # V + ones col per head
            oT_sb = pp.tile([P, DO, N], BF16)     # O.T  [dhg, n] normalized
            nc.vector.memset(v_sb[:, :, :, DH], 1.0)

            with tc.tile_pool(name="psBig", bufs=2, space="PSUM") as psB, \
                 tc.tile_pool(name="psS", bufs=2, space="PSUM") as psS, \
                 tc.tile_pool(name="psPV", bufs=2, space="PSUM") as psPV, \
                 tc.tile_pool(name="pt", bufs=4) as ptp, \
                 tc.tile_pool(name="osb", bufs=2) as osbp, \
                 tc.tile_pool(name="rec", bufs=8) as recp, \
                 tc.tile_pool(name="od", bufs=4) as odp:
                def qproj_pieces(qb, do):
                    # two ~450ns PE pieces sharing one accumulation tile
                    hold = {}

                    def run_a():
                        hold["t"] = psB.tile([P, 512], F32, tag="big", name="qps")
                        for ko in range(4):
                            nc.tensor.matmul(
                                hold["t"][:],
                                wq_sb[:, ko, do * P:(do + 1) * P],
                                xT[:, ko, qb * 512:(qb + 1) * 512],
                                start=(ko == 0), stop=False,
                            )

                    def run_b():
                        for ko in range(4, KO):
                            nc.tensor.matmul(
                                hold["t"][:],
                                wq_sb[:, ko, do * P:(do + 1) * P],
                                xT[:, ko, qb * 512:(qb + 1) * 512],
                                start=False, stop=(ko == KO - 1),
                            )
                        nc.vector.tensor_copy(
                            qT[:, do, qb * 512:(qb + 1) * 512], hold["t"][:]
                        )
                    return [run_a, run_b]

                def transpose_pieces(q0, o_sb):
                    def run():
                        o_flat = o_sb[:].rearrange("p a b -> p (a b)")
                        otp = psB.tile([P, 512], F32, tag="big")
                        for d in range(DO):
                            nc.tensor.transpose(
                                otp[:, d * P:(d + 1) * P],
                                o_flat[:, d * P:(d + 1) * P],
                                ident[:],
                            )
                        nc.vector.tensor_copy(
                            oT_sb[:, :, q0:q0 + P],
                            otp[:].rearrange("p (a b) -> p a b", a=DO),
                        )
                    return [run]

                def proj_pieces(q0, ec):
                    hold = {}

                    def run_a():
                        hold["t"] = psB.tile([P, 512], F32, tag="big", name="fps")
                        for do in range(2):
                            nc.tensor.matmul(
                                hold["t"][:],
                                oT_sb[:, do, q0:q0 + P],
                                wo_sb[:, do, ec * 512:(ec + 1) * 512],
                                start=(do == 0), stop=False,
                            )

                    def run_b():
                        for do in range(2, DO):
                            nc.tensor.matmul(
                                hold["t"][:],
                                oT_sb[:, do, q0:q0 + P],
                                wo_sb[:, do, ec * 512:(ec + 1) * 512],
                                start=False, stop=(do == DO - 1),
                            )
                        ot = odp.tile([P, 512], F32, tag="otile")
                        nc.vector.tensor_tensor(
                            ot[:], hold["t"][:], bo_sb[:, ec * 512:(ec + 1) * 512],
                            mybir.AluOpType.add,
                        )
                        nc.sync.dma_start(
                            out_d[q0:q0 + P, ec * 512:(ec + 1) * 512], ot[:]
                        )
                    return [run_a, run_b]

                # ---------------- K.T projection ------------------------
                for do in range(DO):
                    for ms in range(2):
                        kps = psB.tile([P, 512], F32, tag="big")
                        for ko in range(KO):
                            nc.tensor.matmul(
                                kps[:],
                                wk_sb[:, ko, do * P:(do + 1) * P],
                                ctxT[:, ko, ms * 512:(ms + 1) * 512],
                                start=(ko == 0), stop=(ko == KO - 1),
                            )
                        nc.vector.tensor_copy(kT[:, do, ms * 512:(ms + 1) * 512], kps[:])
                # Q.T for block 0 (so exps can start before V is ready)
                for do in range(DO):
                    for piece in qproj_pieces(0, do):
                        piece()
                # ---------------- V projection --------------------------
                for mo in range(MT):
                    vps = psB.tile([P, 512], F32, tag="big")
                    for ko in range(KO):
                        nc.tensor.matmul(
                            vps[:],
                            ctxT[:, ko, mo * P:(mo + 1) * P],
                            wv_sb[:, ko, :],
                            start=(ko == 0), stop=(ko == KO - 1),
                        )
                    nc.vector.tensor_copy(
                        v_sb[:, mo, :, 0:DH],
                        vps[:].rearrange("p (h d) -> p h d", h=HL),
                    )

                # ------ attention: per q-tile, finalize work of the ------
                # ------ previous q-tile interleaved into the head loop ---
                pending = []
                for qt in range(NT):
                    q0 = qt * P
                    o_sb = osbp.tile([P, HL, DH], F32, tag="osb")
                    for h in range(HL):
                        pb = (h % 2) * DH
                        sps = psS.tile([P, MT, P], F32, tag="s")
                        for mo in range(MT):
                            nc.tensor.matmul(
                                sps[:, mo],
                                kT[pb:pb + DH, h // 2, mo * P:(mo + 1) * P],
                                qT[pb:pb + DH, h // 2, q0:q0 + P],
                                start=True, stop=True,
                                skip_group_check=True,
                            )
                        if pending and h > 6:
                            pending.pop(0)()
                        if len(pending) >= 8:
                            pending.pop(0)()
                        ptile = ptp.tile([P, MT, P], BF16, tag="pt")
                        nc.scalar.activation(
                            ptile[:], sps[:],
                            mybir.ActivationFunctionType.Exp, scale=SCALE,
                        )
                        pv = psPV.tile([P, DH + 1], F32, tag="pv")
                        for mo in range(MT):
                            nc.tensor.matmul(
                                pv[:],
                                ptile[:, mo],
                                v_sb[:, mo, h, :],
                                start=(mo == 0), stop=(mo == MT - 1),
                                skip_group_check=True,
                            )
                        rec = recp.tile([P, 1], F32, tag="rec")
                        nc.vector.reciprocal(rec[:], pv[:, DH:DH + 1])
                        nc.vector.tensor_scalar_mul(o_sb[:, h, :], pv[:, 0:DH], rec[:])
                    pending.extend(transpose_pieces(q0, o_sb))
                    pending.extend(proj_pieces(q0, 0))
                    pending.extend(proj_pieces(q0, 1))
                    if qt % 4 == 1 and qt < 12:
                        # Q.T for the next 512-query block, ahead of its use
                        for do in range(DO):
                            pending.extend(qproj_pieces(qt // 4 + 1, do))
                for t in pending:
                    t()
    nc.finalize()
    return nc


def _get_nc():
    if "nc" not in _CACHE:
        _CACHE["nc"] = _build()
    return _CACHE["nc"]


def kernel(x, context, Wq, Wk, Wv, Wo, bo, **extra):
    nc = _get_nc()
    B = x.shape[0]
    bf = ml_dtypes.bfloat16
    ident = np.eye(P, dtype=np.float32)
    bo_b = np.broadcast_to(np.asarray(bo, dtype=np.float32), (P, E)).copy()
    zeros_bo = np.zeros((P, E), dtype=np.float32)
    x_b = np.asarray(x, dtype=bf)
    ctx_b = np.asarray(context, dtype=bf)
    wq_b = np.asarray(Wq, dtype=bf)
    wk_b = np.asarray(Wk, dtype=bf)
    wv_b = np.asarray(Wv, dtype=bf)
    wo_b = np.asarray(Wo, dtype=bf)
    in_maps = []
    for c in range(8):
        b, g = c // 2, c % 2
        in_maps.append({
            "x": np.ascontiguousarray(x_b[b]),
            "ctx": np.ascontiguousarray(ctx_b[b]),
            "wq": np.ascontiguousarray(wq_b[:, g * DHG:(g + 1) * DHG]),
            "wk": np.ascontiguousarray(wk_b[:, g * DHG:(g + 1) * DHG]),
            "wv": np.ascontiguousarray(wv_b[:, g * DHG:(g + 1) * DHG]),
            "wo": np.ascontiguousarray(wo_b[g * DHG:(g + 1) * DHG, :]),
            "bo": (bo_b if g == 0 else zeros_bo),
            "ident": ident,
        })
    global _last_in_maps
    _last_in_maps = in_maps
    res = run_bass_kernel_spmd(nc, in_maps, list(range(8)))
    out = np.empty((B, N, E), dtype=np.float32)
    for b in range(B):
        out[b] = res.results[2 * b]["out"] + res.results[2 * b + 1]["out"]
    return out


# revision 34
# speedup vs baseline: 1.0731x; 1.0103x over previous
"""Cross-attention Trainium2 kernel (8 NeuronCores, SPMD).

Sharding: core c handles batch c//2 and head-group c%2 (8 of 16 heads).
Each core computes its head-group's partial output projection; the host
sums the two partials per batch (bias is folded into head-group 0).

Shapes (hardcoded): B=4, N=2048 (queries), M=1024 (context), K=1024
(query/context dim), H=16 heads, DH=64, head-group width DHG=512, E=1024.

Dataflow (bf16 compute, fp32 PSUM accumulation / fp32 output):
  host pre-casts x/ctx/weights to bf16 and pre-broadcasts bias.
  xT/ctxT loaded via XBAR DMA transpose (no PE transposes for inputs).
  K.T = Wk.T @ ctxT, V = ctxT.T @ Wv, Q.T = Wq.T @ xT (bf16 matmuls).
  Per (q-tile of 128, head): S.T[m,q] (8 matmuls, d=64), P.T = exp on ACT
  (one 1024-wide activation per head), PV in [q-part, 65] orientation
  (V plus ones column -> softmax row sums land in column 64), reciprocal +
  per-partition-scalar normalize on DVE, O per q-tile PE-transposed back
  to O.T, out = O.T.T @ Wo; bias added by DVE during the PSUM->SBUF copy.
  The finalize work of each q-tile (transpose, output projection, store)
  and the next block's Q.T projection are split into ~450ns pieces and
  interleaved one-per-head into the following q-tile's S/exp/PV loop so
  PE stays busy during the exp latency of the S->exp->PV chain.
"""
import sys

if "/opt/trn_rl_repo" not in sys.path:
    sys.path.insert(0, "/opt/trn_rl_repo")

import numpy as np
import ml_dtypes

import concourse.bass as bass  # noqa: F401
import concourse.tile as tile
from concourse import bacc, mybir
from concourse.bass_utils import run_bass_kernel_spmd

P = 128
N = 2048          # queries per batch
M = 1024          # context rows
K = 1024          # query_dim == context_dim
DHG = 512         # d_attn per head group (8 heads x 64)
DH = 64           # dim per head
HL = 8            # heads per core
E = 1024          # output dim
SCALE = DH ** -0.5
F32 = mybir.dt.float32
BF16 = mybir.dt.bfloat16

KO = K // P       # 8 contraction chunks
NT = N // P       # 16 query tiles
MT = M // P       # 8 context tiles
DO = DHG // P     # 4 head-dim chunks
QC = N // 512     # 4 query blocks of 512
EC = E // 512     # 2 output chunks of 512

_CACHE = {}


def _build():
    nc = bacc.Bacc("TRN2", target_bir_lowering=False, debug=False, num_devices=8)
    x_d = nc.dram_tensor("x", [N, K], BF16, kind="ExternalInput")
    ctx_d = nc.dram_tensor("ctx", [M, K], BF16, kind="ExternalInput")
    wq_d = nc.dram_tensor("wq", [K, DHG], BF16, kind="ExternalInput")
    wk_d = nc.dram_tensor("wk", [K, DHG], BF16, kind="ExternalInput")
    wv_d = nc.dram_tensor("wv", [K, DHG], BF16, kind="ExternalInput")
    wo_d = nc.dram_tensor("wo", [DHG, E], BF16, kind="ExternalInput")
    bo_d = nc.dram_tensor("bo", [P, E], F32, kind="ExternalInput")
    id_d = nc.dram_tensor("ident", [P, P], F32, kind="ExternalInput")
    out_d = nc.dram_tensor("out", [N, E], F32, kind="ExternalOutput")

    with tile.TileContext(nc) as tc:
        with tc.tile_pool(name="persist", bufs=1) as pp:
            # DMA issue order tuned so K-proj (wk+ctxT) then Q-proj (wq+xT0)
            # inputs arrive first on the serialized DMA device.
            ident = pp.tile([P, P], F32)
            bo_sb = pp.tile([P, E], F32)
            wq_sb = pp.tile([P, KO, DHG], BF16)
            wk_sb = pp.tile([P, KO, DHG], BF16)
            wv_sb = pp.tile([P, KO, DHG], BF16)
            wo_sb = pp.tile([P, DO, E], BF16)
            ctxT = pp.tile([P, KO, M], BF16)
            xT = pp.tile([P, KO, N], BF16)

            nc.sync.dma_start(wk_sb[:], wk_d.rearrange("(ko p) d -> p ko d", p=P))
            for ms in range(2):
                nc.sync.dma_start_transpose(
                    ctxT[:, :, ms * 512:(ms + 1) * 512],
                    ctx_d[ms * 512:(ms + 1) * 512, :],
                )
            nc.sync.dma_start(wq_sb[:], wq_d.rearrange("(ko p) d -> p ko d", p=P))
            nc.sync.dma_start_transpose(xT[:, :, 0:512], x_d[0:512, :])
            nc.sync.dma_start(wv_sb[:], wv_d.rearrange("(ko p) d -> p ko d", p=P))
            nc.sync.dma_start(wo_sb[:], wo_d.rearrange("(do p) e -> p do e", p=P))
            for ns in range(1, QC):
                nc.sync.dma_start_transpose(
                    xT[:, :, ns * 512:(ns + 1) * 512],
                    x_d[ns * 512:(ns + 1) * 512, :],
                )
            nc.sync.dma_start(ident[:], id_d[:])
            nc.sync.dma_start(bo_sb[:], bo_d[:])

            kT = pp.tile([P, DO, M], BF16)        # K.T  [dhg, m]
            qT = pp.tile([P, DO, N], BF16)        # Q.T  [dhg, n]
            v_sb = pp.tile([P, MT, HL, DH + 1], BF16)  # V + ones col per head
            oT_sb = pp.tile([P, DO, N], BF16)     # O.T  [dhg, n] normalized
            nc.vector.memset(v_sb[:, :, :, DH], 1.0)

            with tc.tile_pool(name="psBig", bufs=2, space="PSUM") as psB, \
                 tc.tile_pool(name="psS", bufs=2, space="PSUM") as psS, \
                 tc.tile_pool(name="psPV", bufs=2, space="PSUM") as psPV, \
                 tc.tile_pool(name="pt", bufs=4) as ptp, \
                 tc.tile_pool(name="osb", bufs=2) as osbp, \
                 tc.tile_pool(name="rec", bufs=8) as recp, \
                 tc.tile_pool(name="od", bufs=4) as odp:
                def qproj_pieces(qb, do):
                    # two ~450ns PE pieces sharing one accumulation tile
                    hold = {}

                    def run_a():
                        hold["t"] = psB.tile([P, 512], F32, tag="big", name="qps")
                        for ko in range(4):
                            nc.tensor.matmul(
                                hold["t"][:],
                                wq_sb[:, ko, do * P:(do + 1) * P],
                                xT[:, ko, qb * 512:(qb + 1) * 512],
                                start=(ko == 0), stop=False,
                            )

                    def run_b():
                        for ko in range(4, KO):
                            nc.tensor.matmul(
                                hold["t"][:],
                                wq_sb[:, ko, do * P:(do + 1) * P],
                                xT[:, ko, qb * 512:(qb + 1) * 512],
                                start=False, stop=(ko == KO - 1),
                            )
                        nc.vector.tensor_copy(
                            qT[:, do, qb * 512:(qb + 1) * 512], hold["t"][:]
                        )
                    return [run_a, run_b]

                def transpose_pieces(q0, o_sb):
                    def run():
                        o_flat = o_sb[:].rearrange("p a b -> p (a b)")
                        otp = psB.tile([P, 512], F32, tag="big")
                        for d in range(DO):
                            nc.tensor.transpose(
                                otp[:, d * P:(d + 1) * P],
                                o_flat[:, d * P:(d + 1) * P],
                                ident[:],
                            )
                        nc.vector.tensor_copy(
                            oT_sb[:, :, q0:q0 + P],
                            otp[:].rearrange("p (a b) -> p a b", a=DO),
                        )
                    return [run]

                def proj_pieces(q0, ec):
                    hold = {}

                    def run_a():
                        hold["t"] = psB.tile([P, 512], F32, tag="big", name="fps")
                        for do in range(2):
                            nc.tensor.matmul(
                                hold["t"][:],
                                oT_sb[:, do, q0:q0 + P],
                                wo_sb[:, do, ec * 512:(ec + 1) * 512],
                                start=(do == 0), stop=False,
                            )

                    def run_b():
                        for do in range(2, DO):
                            nc.tensor.matmul(
                                hold["t"][:],
                                oT_sb[:, do, q0:q0 + P],
                                wo_sb[:, do, ec * 512:(ec + 1) * 512],
                                start=False, stop=(do == DO - 1),
                            )
                        ot = odp.tile([P, 512], F32, tag="otile")
                        nc.vector.tensor_tensor(
                            ot[:], hold["t"][:], bo_sb[:, ec * 512:(ec + 1) * 512],
                            mybir.AluOpType.add,
                        )
                        nc.sync.dma_start(
                            out_d[q0:q0 + P, ec * 512:(ec + 1) * 512], ot[:]
                        )
                    return [run_a, run_b]

                # ---------------- K.T projection ------------------------
                for do in range(DO):
                    for ms in range(2):
                        kps = psB.tile([P, 512], F32, tag="big")
                        for ko in range(KO):
                            nc.tensor.matmul(
                                kps[:],
                                wk_sb[:, ko, do * P:(do + 1) * P],
                                ctxT[:, ko, ms * 512:(ms + 1) * 512],
                                start=(ko == 0), stop=(ko == KO - 1),
                            )
                        nc.vector.tensor_copy(kT[:, do, ms * 512:(ms + 1) * 512], kps[:])
                # Q.T for block 0 (so exps can start before V is ready)
                for do in range(DO):
                    for piece in qproj_pieces(0, do):
                        piece()
                # ---------------- V projection --------------------------
                for mo in range(MT):
                    vps = psB.tile([P, 512], F32, tag="big")
                    for ko in range(KO):
                        nc.tensor.matmul(
                            vps[:],
                            ctxT[:, ko, mo * P:(mo + 1) * P],
                            wv_sb[:, ko, :],
                            start=(ko == 0), stop=(ko == KO - 1),
                        )
                    nc.vector.tensor_copy(
                        v_sb[:, mo, :, 0:DH],
                        vps[:].rearrange("p (h d) -> p h d", h=HL),
                    )

                # ------ attention: per q-tile, finalize work of the ------
                # ------ previous q-tile interleaved into the head loop ---
                pending = []
                for qt in range(NT):
                    q0 = qt * P
                    o_sb = osbp.tile([P, HL, DH], F32, tag="osb")
                    for h in range(HL):
                        pb = (h % 2) * DH
                        sps = psS.tile([P, MT, P], F32, tag="s")
                        for mo in range(MT):
                            nc.tensor.matmul(
                                sps[:, mo],
                                kT[pb:pb + DH, h // 2, mo * P:(mo + 1) * P],
                                qT[pb:pb + DH, h // 2, q0:q0 + P],
                                start=True, stop=True,
                                skip_group_check=True,
                            )
                        if pending and h > 7:
                            pending.pop(0)()
                        if len(pending) >= 8:
                            pending.pop(0)()
                        ptile = ptp.tile([P, MT, P], BF16, tag="pt")
                        nc.scalar.activation(
                            ptile[:], sps[:],
                            mybir.ActivationFunctionType.Exp, scale=SCALE,
                        )
                        pv = psPV.tile([P, DH + 1], F32, tag="pv")
                        for mo in range(MT):
                            nc.tensor.matmul(
                                pv[:],
                                ptile[:, mo],
                                v_sb[:, mo, h, :],
                                start=(mo == 0), stop=(mo == MT - 1),
                                skip_group_check=True,
                            )
                        rec = recp.tile([P, 1], F32, tag="rec")
                        nc.vector.reciprocal(rec[:], pv[:, DH:DH + 1])
                        nc.vector.tensor_scalar_mul(o_sb[:, h, :], pv[:, 0:DH], rec[:])
                    pending.extend(transpose_pieces(q0, o_sb))
                    pending.extend(proj_pieces(q0, 0))
                    pending.extend(proj_pieces(q0, 1))
                    if qt % 4 == 1 and qt < 12:
                        # Q.T for the next 512-query block, ahead of its use
                        for do in range(DO):
                            pending.extend(qproj_pieces(qt // 4 + 1, do))
                for t in pending:
                    t()
    nc.finalize()
    return nc


def _get_nc():
    if "nc" not in _CACHE:
        _CACHE["nc"] = _build()
    return _CACHE["nc"]


def kernel(x, context, Wq, Wk, Wv, Wo, bo, **extra):
    nc = _get_nc()
    B = x.shape[0]
    bf = ml_dtypes.bfloat16
    ident = np.eye(P, dtype=np.float32)
    bo_b = np.broadcast_to(np.asarray(bo, dtype=np.float32), (P, E)).copy()
    zeros_bo = np.zeros((P, E), dtype=np.float32)
    x_b = np.asarray(x, dtype=bf)
    ctx_b = np.asarray(context, dtype=bf)
    wq_b = np.asarray(Wq, dtype=bf)
    wk_b = np.asarray(Wk, dtype=bf)
    wv_b = np.asarray(Wv, dtype=bf)
    wo_b = np.asarray(Wo, dtype=bf)
    in_maps = []
    for c in range(8):
        b, g = c // 2, c % 2
        in_maps.append({
            "x": np.ascontiguousarray(x_b[b]),
            "ctx": np.ascontiguousarray(ctx_b[b]),
            "wq": np.ascontiguousarray(wq_b[:, g * DHG:(g + 1) * DHG]),
            "wk": np.ascontiguousarray(wk_b[:, g * DHG:(g + 1) * DHG]),
            "wv": np.ascontiguousarray(wv_b[:, g * DHG:(g + 1) * DHG]),
            "wo": np.ascontiguousarray(wo_b[g * DHG:(g + 1) * DHG, :]),
            "bo": (bo_b if g == 0 else zeros_bo),
            "ident": ident,
        })
    global _last_in_maps
    _last_in_maps = in_maps
    res = run_bass_kernel_spmd(nc, in_maps, list(range(8)))
    out = np.empty((B, N, E), dtype=np.float32)
    for b in range(B):
        out[b] = res.results[2 * b]["out"] + res.results[2 * b + 1]["out"]
    return out
